# revision 43
# baseline (speedup 1.0000x reference)
"""Trainium2 Bass kernel for a batch-of-trees BinaryTreeLSTM.

Contract: kernel(**inputs) takes the FULL inputs (B=128 trees, 1023-node
complete binary tree, dim 300) and returns the FULL output (root_c, root_h),
each [128, 300] float32.

Strategy
--------
- Data-parallel over trees: 16 trees per NeuronCore x 8 cores, no collectives.
- Mixed precision GEMMs (measured: a DoubleRow matmul costs 1.0 N-cycles and
  covers TWO 128-row K-chunks, i.e. fp8 is 2x fp16 per K-row; only PLAIN fp8
  wins -- residual-compensated fp8 costs exactly fp16):
  * f, i, o gates (sigmoid, contractive): plain fp8 e4m3 DoubleRow, 3 instrs
    per 601-row contraction (kc01, kc34, kc2+zero-slot) vs 5 for fp16.
    Weights x512 (the 1/512 descale rides the ACT's free affine scale);
    child h / leaf x consumed as plain e4m3 (data errors average out).
  * u gate (tanh, slope 1, sensitive): fp16 weights x fp16 h/x.
  Root rel err ~1.3e-2 (gate 2e-2); sigma-gate weight quantization error is
  systematic but tolerable, h8/x8 data error averages out, u stays clean.
- 13 recurrent M-units: [fL0 fL1 fR0 fR1 T9(fL2|fR2) i0 i1 o0 o1 T8(i2|o2) |
  u0 u1 T12(u2)], tails at partition offsets 0/64; 8 leaf units (io fp8,
  u fp16).  sigma units are contiguous so tail blocks (PB<=256) merge all
  of them into one PSUM group / one wide ACT.
- States per level stored E|O-split, written in place by the gate element-
  wise ops: c16/h16 [128, 5, R/2] (slots E0 E1 O0 O1 t2p), h8 [128, 3, 2,
  R/2] (DR pairs (E0,E1),(O0,O1),(t2p, zeros)); h16 feeds only the u GEMM.
- Every block is software-pipelined into A (GEMMs + gate ACTs + c chain) and
  B (tanh-c + h stores), emitted A(n), B(n-1): the per-block cross-engine
  round trip hides under the next block's A.  h8 stores precede h16 so the
  next level's fp8 GEMM starts earliest in the serial deep-tail levels.
- Levels 1 and 2 are cascaded into the leaf-block loop so their state tiles
  are short-lived pools (SBUF would not fit persistent level-1 state).
- Baseline 353.2 us -> 343.8 us; engines: PE ~249 us busy, DVE ~225,
  ACT ~232.  Leaf phase is ACT-bound, recurrent big blocks PE-bound; the
  rest is startup (~18 us) + the inherently serial deep-tail levels (~32 us,
  each a GEMM-issue + ACT + DVE + tanh round trip on <=256 columns).
"""

import os
import sys

for _p in ("/opt/trn_rl_repo",):
    if os.path.isdir(_p) and _p not in sys.path:
        sys.path.insert(0, _p)

import numpy as np
from contextlib import ExitStack

import concourse.bass as bass
import concourse.tile as tile
from concourse import bacc, mybir
from concourse.bass_utils import run_bass_kernel_spmd

# ---------------------------------------------------------------- constants
N_CORES = 8
B = 128
B_LOC = B // N_CORES          # 16 trees per core
N_LEAVES = 512
MEM = 300
XCOLS = N_LEAVES * B_LOC      # 8192 leaf columns per core
LB = 1024                     # leaf-block columns (64 leaves)
NF = 512                      # block moving dim
R_LVL = {l: XCOLS >> l for l in range(1, 10)}

F16 = mybir.dt.float16
F32 = mybir.dt.float32
F8 = mybir.dt.float8e4
AF = mybir.ActivationFunctionType
SIG = AF.Sigmoid
TANH = AF.Tanh
MUL = mybir.AluOpType.mult
ADD = mybir.AluOpType.add
DR = mybir.MatmulPerfMode.DoubleRow
SW = 512.0                    # fp8 weight scale (descaled in ACT)

# fp8 units (f, i, o) -> Wcat column ranges (Wcat = [i o u fL fR] = 1500)
REC8_UNITS = [
    [(0, 128, 900)], [(0, 128, 1028)],       # fL0 fL1
    [(0, 128, 1200)], [(0, 128, 1328)],      # fR0 fR1
    [(0, 44, 1156), (64, 108, 1456)],        # T9 = fL2 | fR2
    [(0, 128, 0)], [(0, 128, 128)],          # i0 i1
    [(0, 128, 300)], [(0, 128, 428)],        # o0 o1
    [(0, 44, 256), (64, 108, 556)],          # T8 = i2 | o2
]
# G gate-column positions (unit index within the 13-wide G tile)
GP_F = 0        # fL01 at 0:2, fR01 at 2:4
GP_T9 = 4
GP_I = 5        # i01 at 5:7
GP_O = 7        # o01 at 7:9
GP_T8 = 9
GP_U = 10       # u01 at 10:12
GP_T12 = 12
# fp16 units (u only)
REC16_UNITS = [
    [(0, 128, 600)], [(0, 128, 728)],        # u0 u1
    [(0, 44, 856)],                          # T12 = u2
]
LEAF8_UNITS = [
    [(0, 128, 0)], [(0, 128, 128)],          # i0 i1
    [(0, 128, 300)], [(0, 128, 428)],        # o0 o1
    [(0, 44, 256), (64, 108, 556)],          # T6 = i2 | o2
]
LEAF16_UNITS = [
    [(0, 128, 600)], [(0, 128, 728)],        # u0 u1
    [(0, 44, 856)],                          # T7 = u2
]


# ---------------------------------------------------------------- host packing
def _q8f(x):
    import ml_dtypes
    return (np.asarray(x, np.float32)
            .astype(ml_dtypes.float8_e4m3fn).astype(np.float32))


def _pack_weights(Wfioux, b_fioux, Wiouh, Wfh):
    """Returns wrec8 [128, 10*3*256] f8, wrec16 [128, 3*5*128] f16,
    wleaf8 [128, 5*2*256] f8, wleaf16 [128, 3*3*128] f16."""
    import ml_dtypes
    f4 = np.float32
    E4 = ml_dtypes.float8_e4m3fn

    Wcat = np.concatenate([Wiouh, Wfh], axis=1).astype(f4)  # [600, 1500]
    bf = np.asarray(b_fioux, f4)
    bias_cat = np.concatenate(
        [bf[300:600], bf[600:900], bf[900:1200], bf[0:300], bf[0:300]])
    A = _q8f(SW * Wcat)
    Ab = _q8f(SW * bias_cat)

    # wrec8: [p, unit(10), pair(3), two(2), m(128)]
    wrec8 = np.zeros((128, 10, 3, 2, 128), f4)
    for u, cols in enumerate(REC8_UNITS):
        for (m0, m1, c0) in cols:
            w = m1 - m0
            wrec8[:, u, 0, 0, m0:m1] = A[0:128, c0:c0 + w]
            wrec8[:, u, 0, 1, m0:m1] = A[128:256, c0:c0 + w]
            wrec8[:, u, 1, 0, m0:m1] = A[300:428, c0:c0 + w]
            wrec8[:, u, 1, 1, m0:m1] = A[428:556, c0:c0 + w]
            wrec8[0:44, u, 2, 0, m0:m1] = A[256:300, c0:c0 + w]
            wrec8[44, u, 2, 0, m0:m1] = Ab[c0:c0 + w]
            wrec8[64:108, u, 2, 0, m0:m1] = A[556:600, c0:c0 + w]
            # pair2 slot1 stays zero (rhs slot is a zero-padded band)

    # wrec16: [p, unit(3), kc(5), m(128)]; kc = (E0, E1, t2p, O0, O1)
    wrec16 = np.zeros((128, 3, 5, 128), f4)
    for u, cols in enumerate(REC16_UNITS):
        for (m0, m1, c0) in cols:
            w = m1 - m0
            wrec16[:, u, 0, m0:m1] = Wcat[0:128, c0:c0 + w]
            wrec16[:, u, 1, m0:m1] = Wcat[128:256, c0:c0 + w]
            wrec16[0:44, u, 2, m0:m1] = Wcat[256:300, c0:c0 + w]
            wrec16[44, u, 2, m0:m1] = bias_cat[c0:c0 + w]
            wrec16[64:108, u, 2, m0:m1] = Wcat[556:600, c0:c0 + w]
            wrec16[:, u, 3, m0:m1] = Wcat[300:428, c0:c0 + w]
            wrec16[:, u, 4, m0:m1] = Wcat[428:556, c0:c0 + w]

    Wl = np.asarray(Wfioux, f4)[:, 300:1200]     # [300, 900]
    bl = bf[300:1200]
    Al = _q8f(SW * Wl)
    Abl = _q8f(SW * bl)

    # wleaf8: [p, unit(5), pair(2), two(2), m(128)]; pair1 = (kc2, zeros)
    wleaf8 = np.zeros((128, 5, 2, 2, 128), f4)
    for u, cols in enumerate(LEAF8_UNITS):
        for (m0, m1, c0) in cols:
            w = m1 - m0
            wleaf8[:, u, 0, 0, m0:m1] = Al[0:128, c0:c0 + w]
            wleaf8[:, u, 0, 1, m0:m1] = Al[128:256, c0:c0 + w]
            wleaf8[0:44, u, 1, 0, m0:m1] = Al[256:300, c0:c0 + w]
            wleaf8[44, u, 1, 0, m0:m1] = Abl[c0:c0 + w]

    # wleaf16: [p, unit(3), kc(3), m(128)]
    wleaf16 = np.zeros((128, 3, 3, 128), f4)
    for u, cols in enumerate(LEAF16_UNITS):
        for (m0, m1, c0) in cols:
            w = m1 - m0
            wleaf16[:, u, 0, m0:m1] = Wl[0:128, c0:c0 + w]
            wleaf16[:, u, 1, m0:m1] = Wl[128:256, c0:c0 + w]
            wleaf16[0:44, u, 2, m0:m1] = Wl[256:300, c0:c0 + w]
            wleaf16[44, u, 2, m0:m1] = bl[c0:c0 + w]

    return (wrec8.reshape(128, -1).astype(E4),
            wrec16.reshape(128, -1).astype(np.float16),
            wleaf8.reshape(128, -1).astype(E4),
            wleaf16.reshape(128, -1).astype(np.float16))


def _check_topology(left_idx, right_idx, leaf_mask):
    li = np.asarray(left_idx); ri = np.asarray(right_idx)
    prev = np.arange(N_LEAVES); nid = N_LEAVES
    ok = bool((np.asarray(leaf_mask)[:N_LEAVES] == 1).all())
    ok &= bool((np.asarray(leaf_mask)[N_LEAVES:] == 0).all())
    while len(prev) > 1:
        cur = []
        for k in range(0, len(prev), 2):
            ok &= bool(li[nid] == prev[k]) and bool(ri[nid] == prev[k + 1])
            cur.append(nid); nid += 1
        prev = np.asarray(cur)
    return ok


# ---------------------------------------------------------------- bass program
def build_program():
    nc = bacc.Bacc("TRN2", target_bir_lowering=False, debug=False)

    xt_d = nc.dram_tensor("xt", [128, 2, XCOLS], F16, kind="ExternalInput").ap()
    x2t_d = nc.dram_tensor("x2t", [44, XCOLS], F16, kind="ExternalInput").ap()
    x8t_d = nc.dram_tensor("x8t", [128, 4, XCOLS], F8,
                           kind="ExternalInput").ap()
    wrec8_d = nc.dram_tensor("wrec8", [128, 10 * 3 * 256], F8,
                             kind="ExternalInput").ap()
    wrec16_d = nc.dram_tensor("wrec16", [128, 3 * 5 * 128], F16,
                              kind="ExternalInput").ap()
    wleaf8_d = nc.dram_tensor("wleaf8", [128, 5 * 2 * 256], F8,
                              kind="ExternalInput").ap()
    wleaf16_d = nc.dram_tensor("wleaf16", [128, 3 * 3 * 128], F16,
                               kind="ExternalInput").ap()
    cons_d = nc.dram_tensor("cons", [84, 2 * LB], F16, kind="ExternalInput").ap()
    cons8_d = nc.dram_tensor("cons8", [84, LB], F8, kind="ExternalInput").ap()
    out_d = nc.dram_tensor("out", [128, 6 * B_LOC], F16,
                           kind="ExternalOutput").ap()

    with ExitStack() as ctx:
        tc = ctx.enter_context(tile.TileContext(nc))
        _build(ctx, tc, xt_d, x2t_d, x8t_d, wrec8_d, wrec16_d, wleaf8_d,
               wleaf16_d, cons_d, cons8_d, out_d)

    nc.compile()
    return nc


def _build(ctx, tc, xt_d, x2t_d, x8t_d, wrec8_d, wrec16_d, wleaf8_d,
           wleaf16_d, cons_d, cons8_d, out_d):
    nc = tc.nc

    wpool = ctx.enter_context(tc.tile_pool(name="wpool", bufs=1))
    state_pool = ctx.enter_context(tc.tile_pool(name="state", bufs=1))

    # ---- weights resident in SBUF (leaf weights first: needed immediately)
    wleaf16_t = wpool.tile([128, 3, 3, 128], F16, name="wleaf16")
    nc.sync.dma_start(wleaf16_t[:], wleaf16_d[:].rearrange(
        "p (u kc m) -> p u kc m", u=3, kc=3))
    wleaf8_t = wpool.tile([128, 5, 2, 2, 128], F8, name="wleaf8")
    _wl8 = wleaf8_d[:].rearrange("p (u pr two m) -> p u pr two m",
                                 u=5, pr=2, two=2)
    nc.sync.dma_start(wleaf8_t[:, 0:4], _wl8[:, 0:4])
    nc.sync.dma_start(wleaf8_t[:, 4:5], _wl8[:, 4:5])
    wrec8_t = wpool.tile([128, 10, 3, 2, 128], F8, name="wrec8")
    wrec16_t = wpool.tile([128, 3, 5, 128], F16, name="wrec16")

    # ---- persistent state for levels 2..8: c16/h16 [128, 5, R/2] slots
    # (E0 E1 O0 O1 t2p), h8 [128, 3, 2, R/2] pairs ((E0,E1),(O0,O1),(t2p,0))
    ST = {}
    for lvl in range(2, 9):
        R = R_LVL[lvl]
        ST[lvl] = dict(
            c=state_pool.tile([128, 5, R // 2], F16, name=f"c_{lvl}"),
            h16=state_pool.tile([128, 5, R // 2], F16, name=f"h16_{lvl}"),
            h8=state_pool.tile([128, 3, 2, R // 2], F8, name=f"h8_{lvl}"),
            R=R)

    # leaf cells (per leaf block, 2 bufs) and L1 cells (1024 L1-cols each)
    LC = [dict(c=state_pool.tile([128, 5, LB // 2], F16, name=f"lc{i}"),
               h16=state_pool.tile([128, 5, LB // 2], F16, name=f"lh16{i}"),
               h8=state_pool.tile([128, 3, 2, LB // 2], F8, name=f"lh8{i}"),
               R=LB) for i in range(2)]
    C1 = [dict(c=state_pool.tile([128, 5, NF], F16, name=f"c1_{i}"),
               h16=state_pool.tile([128, 5, NF], F16, name=f"h16_1{i}"),
               h8=state_pool.tile([128, 3, 2, NF], F8, name=f"h8_1{i}"),
               R=2 * NF) for i in range(2)]

    # persistent x2 (fp16 leaf tail chunk with bias/zero rows)
    x2_t = [state_pool.tile([128, LB], F16, name=f"x2_{i}") for i in range(2)]

    def _pad_cell(cell):
        W = cell["R"] // 2
        nc.sync.dma_start(cell["h16"][44:64, 4, :], cons_d[0:20, :W])
        nc.sync.dma_start(cell["h16"][108:128, 4, :], cons_d[1:21, :W])
        nc.sync.dma_start(cell["c"][44:64, 4, :], cons_d[1:21, :W])
        nc.sync.dma_start(cell["c"][108:128, 4, :], cons_d[1:21, :W])
        nc.sync.dma_start(cell["h8"][44:64, 2, 0, :], cons8_d[0:20, :W])
        nc.sync.dma_start(cell["h8"][108:128, 2, 0, :], cons8_d[1:21, :W])
        # pair-2 slot 1: fully zero (matching weight slot is zero too)
        nc.sync.dma_start(cell["h8"][0:64, 2, 1, :], cons8_d[1:65, :W])
        nc.sync.dma_start(cell["h8"][64:128, 2, 1, :], cons8_d[1:65, :W])

    def _pad_x2():
        for i in range(2):
            nc.sync.dma_start(x2_t[i][44:128, :], cons_d[0:84, :LB])

    def _pad_dmas_early():
        for cell in LC + C1:
            _pad_cell(cell)

    def _pad_dmas_late():
        for lvl in range(2, 9):
            _pad_cell(ST[lvl])

    # ---- pools
    xpool = ctx.enter_context(tc.tile_pool(name="xpool", bufs=2))
    glpool = ctx.enter_context(tc.tile_pool(name="gl", bufs=2))
    gpool = ctx.enter_context(tc.tile_pool(name="g", bufs=2))
    pspool = ctx.enter_context(tc.tile_pool(name="ps", bufs=2, space="PSUM"))
    thpool = ctx.enter_context(tc.tile_pool(name="th", bufs=2))
    tpool = ctx.enter_context(tc.tile_pool(name="t", bufs=2))
    opool = ctx.enter_context(tc.tile_pool(name="o", bufs=1))

    # ================================================================ helpers
    def fp8_group(ps, units, rhs_pairs, PB):
        """Plain-fp8 DoubleRow GEMMs: 3 pair-instrs per unit."""
        for j, u in enumerate(units):
            for k in range(3):
                nc.tensor.matmul(ps[:, j * PB:(j + 1) * PB],
                                 wrec8_t[:, u, k, :, :], rhs_pairs[k],
                                 start=(k == 0), stop=(k == 2),
                                 perf_mode=DR)

    def fp16_group(ps, j0, units, wt, rhs_chunks, PB):
        nkc = len(rhs_chunks)
        for j, u in enumerate(units):
            for k in range(nkc):
                nc.tensor.matmul(ps[:, (j0 + j) * PB:(j0 + j + 1) * PB],
                                 wt[:, u, k, :], rhs_chunks[k],
                                 start=(k == 0), stop=(k == nkc - 1))

    def rec_block(lvl, q0, PB, prev, pq0, dst):
        """One recurrent block: cols q0:q0+PB at level lvl; children at
        E/O positions pq0:pq0+PB of `prev`; dst = cell dict or "root".

        Emits the A-part (GEMMs, gate ACTs, c-chain DVE) inline and returns
        the B-part (tanh-c ACTs + h stores) as a closure, so the caller can
        software-pipeline B behind the next block's A."""
        hw = PB // 2
        h8p = prev["h8"]
        rhs_pairs = [h8p[:, 0, :, pq0:pq0 + PB], h8p[:, 1, :, pq0:pq0 + PB],
                     h8p[:, 2, :, pq0:pq0 + PB]]
        h16p = prev["h16"]
        rhs16 = [h16p[:, 0, pq0:pq0 + PB], h16p[:, 1, pq0:pq0 + PB],
                 h16p[:, 4, pq0:pq0 + PB], h16p[:, 2, pq0:pq0 + PB],
                 h16p[:, 3, pq0:pq0 + PB]]

        G = gpool.tile([128, 13 * NF], F16, tag="G", name=f"G{lvl}")

        # G layout: [fL0 fL1 fR0 fR1 T9 i0 i1 o0 | o1 T8 | u0 u1 T12]
        # (sigma units contiguous at 0:10, tanh at 10:13)
        if PB > 256:
            ps1 = pspool.tile([128, 4 * NF], F32, tag="ps", name="ps1")
            fp8_group(ps1[:, :4 * PB], (0, 1, 2, 3), rhs_pairs, PB)
            ps2 = pspool.tile([128, 4 * NF], F32, tag="ps", name="ps2")
            fp8_group(ps2[:, :4 * PB], (4, 5, 6, 7), rhs_pairs, PB)
            nc.scalar.activation(G[:, 0:4 * PB], ps1[:, :4 * PB], SIG,
                                 scale=1.0 / SW)
            ps3 = pspool.tile([128, 4 * NF], F32, tag="ps", name="ps3")
            fp8_group(ps3[:, :2 * PB], (8, 9), rhs_pairs, PB)
            fp16_group(ps3, 2, (0, 1), wrec16_t, rhs16, PB)
            nc.scalar.activation(G[:, 4 * PB:8 * PB], ps2[:, :4 * PB], SIG,
                                 scale=1.0 / SW)
            ps4 = pspool.tile([128, 4 * NF], F32, tag="ps", name="ps4")
            fp16_group(ps4, 0, (2,), wrec16_t, rhs16, PB)
            nc.scalar.activation(G[:, 8 * PB:10 * PB], ps3[:, 0:2 * PB], SIG,
                                 scale=1.0 / SW)
            nc.scalar.activation(G[:, 10 * PB:12 * PB], ps3[:, 2 * PB:4 * PB],
                                 TANH)
            nc.scalar.activation(G[:, 12 * PB:13 * PB], ps4[:, 0:PB], TANH)
        else:
            # tail blocks: merged groups, fewer ACTs / PSUM round-trips
            n8 = 8 if PB == 256 else 10
            ps1 = pspool.tile([128, 4 * NF], F32, tag="ps", name="ps1")
            fp8_group(ps1[:, :n8 * PB], tuple(range(n8)), rhs_pairs, PB)
            ps2 = pspool.tile([128, 4 * NF], F32, tag="ps", name="ps2")
            j0 = 0
            if n8 == 8:
                fp8_group(ps2[:, :2 * PB], (8, 9), rhs_pairs, PB)
                j0 = 2
            fp16_group(ps2, j0, (0, 1, 2), wrec16_t, rhs16, PB)
            nc.scalar.activation(G[:, 0:n8 * PB], ps1[:, :n8 * PB], SIG,
                                 scale=1.0 / SW)
            if n8 == 8:
                nc.scalar.activation(G[:, 8 * PB:10 * PB], ps2[:, 0:2 * PB],
                                     SIG, scale=1.0 / SW)
            nc.scalar.activation(G[:, 10 * PB:13 * PB],
                                 ps2[:, j0 * PB:(j0 + 3) * PB], TANH)

        # ---- elementwise
        cp = prev["c"]
        c2p = cp[:, 4, pq0:pq0 + PB]           # [p, PB] (E@0:44, O@64:108)

        t1 = tpool.tile([128, 4, NF], F16, tag="t1", name="t1", bufs=1)
        t12a = tpool.tile([64, NF], F16, tag="t12a", name="t12a", bufs=1)
        t12b = tpool.tile([64, NF], F16, tag="t12b", name="t12b", bufs=1)
        fc = tpool.tile([128, 2, NF], F16, tag="fc", name="fc", bufs=1)
        fc2 = tpool.tile([64, NF], F16, tag="fc2", name="fc2", bufs=1)
        iu = tpool.tile([128, 2, NF], F16, tag="iu", name="iu", bufs=1)
        iu2 = tpool.tile([64, NF], F16, tag="iu2", name="iu2", bufs=1)

        g2 = G[:, 0:13 * PB].rearrange("p (u n) -> p u n", u=13)
        nc.vector.tensor_tensor(t1[:, :, :PB], g2[:, 0:4, :],
                                cp[:, 0:4, pq0:pq0 + PB], MUL)
        nc.vector.tensor_tensor(t12a[0:44, :PB], g2[0:44, GP_T9, :],
                                c2p[0:44], MUL)
        nc.vector.tensor_tensor(t12b[0:44, :PB], g2[64:108, GP_T9, :],
                                c2p[64:108], MUL)
        nc.vector.tensor_tensor(fc[:, :, :PB], t1[:, 0:2, :PB],
                                t1[:, 2:4, :PB], ADD)
        nc.vector.tensor_tensor(fc2[0:44, :PB], t12a[0:44, :PB],
                                t12b[0:44, :PB], ADD)
        nc.vector.tensor_tensor(iu[:, :, :PB], g2[:, GP_I:GP_I + 2, :],
                                g2[:, GP_U:GP_U + 2, :], MUL)
        nc.vector.tensor_tensor(iu2[0:44, :PB], g2[0:44, GP_T8, :],
                                g2[0:44, GP_T12, :], MUL)

        if dst == "root":
            ot = opool.tile([128, 6 * B_LOC], F16, name="ot")
            nc.sync.dma_start(ot[44:128, 2 * B_LOC:3 * B_LOC],
                              cons_d[0:84, B_LOC:2 * B_LOC])
            nc.sync.dma_start(ot[44:128, 5 * B_LOC:6 * B_LOC],
                              cons_d[0:84, B_LOC:2 * B_LOC])
            oc = ot[:, 0:2 * B_LOC].rearrange("p (c n) -> p c n", c=2)
            nc.vector.tensor_tensor(oc, iu[:, :, :PB], fc[:, :, :PB], ADD)
            nc.vector.tensor_tensor(ot[0:44, 2 * B_LOC:3 * B_LOC],
                                    iu2[0:44, :PB], fc2[0:44, :PB], ADD)
            tho = thpool.tile([128, 5, NF // 2], F16, tag="th", name="tho")
            nc.scalar.activation(tho[:, 0:2, :PB], oc, TANH)
            nc.scalar.activation(tho[64:108, 2, :PB],
                                 ot[0:44, 2 * B_LOC:3 * B_LOC], TANH)
            oh = ot[:, 3 * B_LOC:5 * B_LOC].rearrange("p (c n) -> p c n", c=2)
            nc.vector.tensor_tensor(oh, g2[:, GP_O:GP_O + 2, :],
                                    tho[:, 0:2, :PB], MUL)
            nc.vector.tensor_tensor(ot[0:44, 5 * B_LOC:6 * B_LOC],
                                    g2[64:108, GP_T8, :],
                                    tho[64:108, 2, :PB], MUL)
            nc.sync.dma_start(out_d[:, :], ot[:, :])
            return None

        qh = q0 // 2
        ct, h16t, h8t = dst["c"], dst["h16"], dst["h8"]
        # c store, E|O split (two ops: ISA allows at most 3 free dims)
        iu4 = iu[:, :, :PB].rearrange("p ch (m two b) -> p two ch m b",
                                      two=2, b=B_LOC)
        fc4 = fc[:, :, :PB].rearrange("p ch (m two b) -> p two ch m b",
                                      two=2, b=B_LOC)
        ce = ct[:, 0:2, qh:qh + hw].rearrange("p c (m b) -> p c m b", b=B_LOC)
        cod = ct[:, 2:4, qh:qh + hw].rearrange("p c (m b) -> p c m b", b=B_LOC)
        nc.vector.tensor_tensor(ce, iu4[:, 0], fc4[:, 0], ADD)
        nc.vector.tensor_tensor(cod, iu4[:, 1], fc4[:, 1], ADD)
        # c tail: interleaved at band [64:108], then E|O copies into the state
        tci = tpool.tile([128, NF], F16, tag="tci", name="tci")
        nc.vector.tensor_tensor(tci[64:108, :PB], iu2[0:44, :PB],
                                fc2[0:44, :PB], ADD)
        tcv = tci[64:108, :PB].rearrange("p (m two b) -> p m two b",
                                         two=2, b=B_LOC)
        c2o = ct[:, 4, qh:qh + hw].rearrange("p (m b) -> p m b", b=B_LOC)
        nc.vector.tensor_scalar_mul(c2o[0:44], tcv[:, :, 0, :], 1.0)
        nc.vector.tensor_scalar_mul(c2o[64:108], tcv[:, :, 1, :], 1.0)

        def finish():
            # tanh: main slots from the state, tail from the interleaved band
            th = thpool.tile([128, 5, NF // 2], F16, tag="th", name="th")
            nc.scalar.activation(th[:, 0:4, :hw], ct[:, 0:4, qh:qh + hw],
                                 TANH)
            th2i = tpool.tile([128, NF], F16, tag="th2i", name="th2i")
            nc.scalar.activation(th2i[64:108, :PB], tci[64:108, :PB], TANH)

            # h stores: h16 (GpSimd, off critical path) and h8 (DVE)
            o4 = g2[:, GP_O:GP_O + 2, :].rearrange(
                "p ch (m two b) -> p two ch m b", two=2, b=B_LOC)
            o2g = g2[64:108, GP_T8, :].rearrange("p (m two b) -> p m two b",
                                                 two=2, b=B_LOC)
            thr = th[:, :, :hw]
            the = thr[:, 0:2, :].rearrange("p ch (m b) -> p ch m b", b=B_LOC)
            tho_ = thr[:, 2:4, :].rearrange("p ch (m b) -> p ch m b", b=B_LOC)
            th2 = th2i[64:108, :PB].rearrange("p (m two b) -> p m two b",
                                              two=2, b=B_LOC)

            h16e = h16t[:, 0:2, qh:qh + hw].rearrange("p c (m b) -> p c m b",
                                                      b=B_LOC)
            h16o = h16t[:, 2:4, qh:qh + hw].rearrange("p c (m b) -> p c m b",
                                                      b=B_LOC)
            h16_2 = h16t[:, 4, qh:qh + hw].rearrange("p (m b) -> p m b",
                                                     b=B_LOC)
            nc.vector.tensor_tensor(h16e, o4[:, 0], the, MUL)
            nc.vector.tensor_tensor(h16o, o4[:, 1], tho_, MUL)
            nc.vector.tensor_tensor(h16_2[0:44], o2g[:, :, 0, :],
                                    th2[:, :, 0, :], MUL)
            nc.vector.tensor_tensor(h16_2[64:108], o2g[:, :, 1, :],
                                    th2[:, :, 1, :], MUL)

            h8e = h8t[:, 0, :, qh:qh + hw].rearrange("p c (m b) -> p c m b",
                                                     b=B_LOC)
            h8o = h8t[:, 1, :, qh:qh + hw].rearrange("p c (m b) -> p c m b",
                                                     b=B_LOC)
            nc.vector.tensor_tensor(h8e, o4[:, 0], the, MUL)
            nc.vector.tensor_tensor(h8o, o4[:, 1], tho_, MUL)
            h8_2 = h8t[:, 2, 0, qh:qh + hw].rearrange("p (m b) -> p m b",
                                                      b=B_LOC)
            nc.vector.tensor_tensor(h8_2[0:44], o2g[:, :, 0, :],
                                    th2[:, :, 0, :], MUL)
            nc.vector.tensor_tensor(h8_2[64:108], o2g[:, :, 1, :],
                                    th2[:, :, 1, :], MUL)

        return finish

    # ---------------------------------------------------------------- leaves
    def leaf_sub(x01, x2, x8, s, cell):
        """Leaf sub-chunk (512 cols): GEMM + gate ACTs + c-chain (A-part);
        returns the B-part closure.  Gl layout: [i0 i1 o0 o1 | T6 u0 u1 T7]"""
        n0 = s * NF
        hw = NF // 2
        qh = s * hw
        rhs16 = [x01[:, 0, n0:n0 + NF], x01[:, 1, n0:n0 + NF],
                 x2[:, n0:n0 + NF]]
        x8p = [x8[:, 0, :, n0:n0 + NF], x8[:, 1, :, n0:n0 + NF]]
        Gl = glpool.tile([128, 8 * NF], F16, tag="Gl", name="Gl")
        psA = pspool.tile([128, 4 * NF], F32, tag="ps", name="lpsA")
        for j, u in enumerate((0, 1, 2, 3)):
            for k in range(2):
                nc.tensor.matmul(psA[:, j * NF:(j + 1) * NF],
                                 wleaf8_t[:, u, k, :, :], x8p[k],
                                 start=(k == 0), stop=(k == 1), perf_mode=DR)
        psB = pspool.tile([128, 4 * NF], F32, tag="ps", name="lpsB")
        for k in range(2):
            nc.tensor.matmul(psB[:, 0:NF], wleaf8_t[:, 4, k, :, :], x8p[k],
                             start=(k == 0), stop=(k == 1), perf_mode=DR)
        fp16_group(psB, 1, (0, 1, 2), wleaf16_t, rhs16, NF)
        nc.scalar.activation(Gl[:, 0:4 * NF], psA[:, :], SIG, scale=1.0 / SW)
        nc.scalar.activation(Gl[:, 4 * NF:5 * NF], psB[:, 0:NF], SIG,
                             scale=1.0 / SW)
        nc.scalar.activation(Gl[:, 5 * NF:8 * NF], psB[:, NF:4 * NF], TANH)

        ct, h16t, h8t = cell["c"], cell["h16"], cell["h8"]
        g2 = Gl.rearrange("p (u n) -> p u n", u=8)
        i4 = g2[:, 0:2, :].rearrange("p ch (m two b) -> p two ch m b",
                                     two=2, b=B_LOC)
        u4 = g2[:, 5:7, :].rearrange("p ch (m two b) -> p two ch m b",
                                     two=2, b=B_LOC)

        ce = ct[:, 0:2, qh:qh + hw].rearrange("p c (m b) -> p c m b", b=B_LOC)
        co = ct[:, 2:4, qh:qh + hw].rearrange("p c (m b) -> p c m b", b=B_LOC)
        nc.vector.tensor_tensor(ce, i4[:, 0], u4[:, 0], MUL)
        nc.vector.tensor_tensor(co, i4[:, 1], u4[:, 1], MUL)
        # interleaved tail c at band [64:108], then E|O copies into the state
        tci = tpool.tile([128, NF], F16, tag="tci", name="ltci")
        nc.vector.tensor_tensor(tci[64:108, :NF], g2[0:44, 4, :],
                                g2[0:44, 7, :], MUL)
        tcv = tci[64:108, :NF].rearrange("p (m two b) -> p m two b",
                                         two=2, b=B_LOC)
        c2 = ct[:, 4, qh:qh + hw].rearrange("p (m b) -> p m b", b=B_LOC)
        nc.vector.tensor_scalar_mul(c2[0:44], tcv[:, :, 0, :], 1.0)
        nc.vector.tensor_scalar_mul(c2[64:108], tcv[:, :, 1, :], 1.0)

        def finish():
            o4 = g2[:, 2:4, :].rearrange("p ch (m two b) -> p two ch m b",
                                         two=2, b=B_LOC)
            o2g = g2[64:108, 4, :].rearrange("p (m two b) -> p m two b",
                                             two=2, b=B_LOC)
            th = thpool.tile([128, 5, NF // 2], F16, tag="th", name="lth")
            nc.scalar.activation(th[:, 0:4, :hw], ct[:, 0:4, qh:qh + hw],
                                 TANH)
            th2i = tpool.tile([128, NF], F16, tag="th2i", name="lth2i")
            nc.scalar.activation(th2i[64:108, :NF], tci[64:108, :NF], TANH)
            the = th[:, 0:2, :hw].rearrange("p c (m b) -> p c m b", b=B_LOC)
            tho_ = th[:, 2:4, :hw].rearrange("p c (m b) -> p c m b", b=B_LOC)
            th2 = th2i[64:108, :NF].rearrange("p (m two b) -> p m two b",
                                              two=2, b=B_LOC)

            h16e = h16t[:, 0:2, qh:qh + hw].rearrange("p c (m b) -> p c m b",
                                                      b=B_LOC)
            h16o = h16t[:, 2:4, qh:qh + hw].rearrange("p c (m b) -> p c m b",
                                                      b=B_LOC)
            h16_2 = h16t[:, 4, qh:qh + hw].rearrange("p (m b) -> p m b",
                                                     b=B_LOC)
            nc.vector.tensor_tensor(h16e, o4[:, 0], the, MUL)
            nc.vector.tensor_tensor(h16o, o4[:, 1], tho_, MUL)
            nc.vector.tensor_tensor(h16_2[0:44], o2g[:, :, 0, :],
                                    th2[:, :, 0, :], MUL)
            nc.vector.tensor_tensor(h16_2[64:108], o2g[:, :, 1, :],
                                    th2[:, :, 1, :], MUL)
            h8e = h8t[:, 0, :, qh:qh + hw].rearrange("p c (m b) -> p c m b",
                                                     b=B_LOC)
            h8o = h8t[:, 1, :, qh:qh + hw].rearrange("p c (m b) -> p c m b",
                                                     b=B_LOC)
            nc.vector.tensor_tensor(h8e, o4[:, 0], the, MUL)
            nc.vector.tensor_tensor(h8o, o4[:, 1], tho_, MUL)
            h8_2 = h8t[:, 2, 0, qh:qh + hw].rearrange("p (m b) -> p m b",
                                                      b=B_LOC)
            nc.vector.tensor_tensor(h8_2[0:44], o2g[:, :, 0, :],
                                    th2[:, :, 0, :], MUL)
            nc.vector.tensor_tensor(h8_2[64:108], o2g[:, :, 1, :],
                                    th2[:, :, 1, :], MUL)

        return finish

    # ================================================================ schedule
    # Software pipeline: emit A(block n) then B(block n-1); B(X) must precede
    # A(any consumer of X) -- guaranteed by the global block order below plus
    # explicit flushes at phase-B level boundaries.
    PEND = [None]

    def sched(b_fn):
        if PEND[0] is not None:
            PEND[0]()
        PEND[0] = b_fn

    def flush():
        if PEND[0] is not None:
            PEND[0]()
            PEND[0] = None

    n_blk = XCOLS // LB                       # 8 leaf blocks
    xtiles = {}

    def dma_x(j):
        c0 = j * LB
        x01 = xpool.tile([128, 2, LB], F16, tag="x01", name="x01")
        x8 = xpool.tile([128, 2, 2, LB], F8, tag="x8", name="x8")
        x2 = x2_t[j % 2]
        h = LB // 2
        for t in range(2):
            nc.sync.dma_start(x8[:, 0, :, t * h:(t + 1) * h],
                              x8t_d[:, 0:2, c0 + t * h:c0 + (t + 1) * h])
            nc.sync.dma_start(x8[:, 1, :, t * h:(t + 1) * h],
                              x8t_d[:, 2:4, c0 + t * h:c0 + (t + 1) * h])
            nc.sync.dma_start(x01[:, :, t * h:(t + 1) * h],
                              xt_d[:, :, c0 + t * h:c0 + (t + 1) * h])
            nc.sync.dma_start(x2[0:44, t * h:(t + 1) * h],
                              x2t_d[:, c0 + t * h:c0 + (t + 1) * h])
        xtiles[j] = (x01, x2, x8)

    _pad_x2()
    dma_x(0)
    for blk in range(n_blk):
        x01, x2, x8 = xtiles.pop(blk)
        if blk + 1 < n_blk:
            dma_x(blk + 1)                    # prefetch next block's x
        if blk == 0:
            _pad_dmas_early()
            nc.sync.dma_start(wrec8_t[:], wrec8_d[:].rearrange(
                "p (u pr two m) -> p u pr two m", u=10, pr=3, two=2))
            nc.sync.dma_start(wrec16_t[:], wrec16_d[:].rearrange(
                "p (u kc m) -> p u kc m", u=3, kc=5))
        elif blk == 2:
            _pad_dmas_late()

        cell = LC[blk % 2]
        for s in range(2):
            sched(leaf_sub(x01, x2, x8, s, cell))
        if blk >= 1:
            j = blk - 1
            # L1 block j: output into C1 cell j//2 at col offset (j%2)*NF
            sched(rec_block(1, (j % 2) * NF, NF, LC[j % 2], 0,
                            C1[(j // 2) % 2]))
        if blk >= 3 and blk % 2 == 1:
            k = (blk - 3) // 2
            sched(rec_block(2, k * NF, NF, C1[k % 2], 0, ST[2]))
    # drain: l1(7), l2(3) -- l2(3) consumes l1(7) so flush in between
    sched(rec_block(1, NF, NF, LC[1], 0, C1[1]))
    flush()
    sched(rec_block(2, 3 * NF, NF, C1[1], 0, ST[2]))
    flush()

    # ---------------------------------------------------------------- phase B
    for lvl in range(3, 10):
        R = R_LVL[lvl]
        PB = min(NF, R)
        prev = ST[lvl - 1]
        for q0 in range(0, R, PB):
            b = rec_block(lvl, q0, PB, prev, q0,
                          ST[lvl] if lvl < 9 else "root")
            if b is not None:
                sched(b)
        flush()


# ---------------------------------------------------------------- runner
_CACHE = {}


def _get_program():
    if "nc" not in _CACHE:
        _CACHE["nc"] = build_program()
    return _CACHE["nc"]


def _host_inputs(inputs, Wfioux, b_fioux, Wiouh, Wfh):
    import ml_dtypes
    E4 = ml_dtypes.float8_e4m3fn
    wrec8, wrec16, wleaf8, wleaf16 = _pack_weights(
        np.asarray(Wfioux, np.float32), np.asarray(b_fioux, np.float32),
        np.asarray(Wiouh, np.float32), np.asarray(Wfh, np.float32))
    cons = np.zeros((84, 2 * LB), np.float16)
    cons[0, :] = 1.0
    cons8 = np.zeros((84, LB), np.float32)
    cons8[0, :] = 1.0
    cons8 = cons8.astype(E4)
    in_maps = []
    for core in range(N_CORES):
        x = np.asarray(inputs[core * B_LOC:(core + 1) * B_LOC, :N_LEAVES, :],
                       np.float32)
        xt_full = x.transpose(2, 1, 0).reshape(MEM, XCOLS)
        xt = np.ascontiguousarray(
            xt_full[0:256].reshape(2, 128, XCOLS).transpose(1, 0, 2)
        ).astype(np.float16)
        x2t = np.ascontiguousarray(xt_full[256:300]).astype(np.float16)
        x8t = np.zeros((128, 4, XCOLS), np.float32)
        x8t[:, 0, :] = xt_full[0:128]
        x8t[:, 1, :] = xt_full[128:256]
        x8t[0:44, 2, :] = xt_full[256:300]
        x8t[44, 2, :] = 1.0
        x8t = x8t.astype(E4)
        in_maps.append({"xt": xt, "x2t": x2t, "x8t": x8t, "wrec8": wrec8,
                        "wrec16": wrec16, "wleaf8": wleaf8,
                        "wleaf16": wleaf16, "cons": cons, "cons8": cons8})
    return in_maps


def kernel(inputs, Wfioux, b_fioux, Wiouh, Wfh, left_idx, right_idx, leaf_mask,
           _trace=False, _trace_dir=None):
    inputs = np.asarray(inputs, np.float32)
    assert _check_topology(left_idx, right_idx, leaf_mask), \
        "tree topology does not match the expected complete binary tree"

    in_maps = _host_inputs(inputs, Wfioux, b_fioux, Wiouh, Wfh)
    nc = _get_program()
    res = run_bass_kernel_spmd(nc, in_maps, list(range(N_CORES)),
                               trace=_trace, tmpdir=_trace_dir)

    root_c = np.zeros((B, MEM), np.float32)
    root_h = np.zeros((B, MEM), np.float32)
    for core in range(N_CORES):
        out = np.asarray(res.results[core]["out"], np.float32)  # [128, 96]
        sl = slice(core * B_LOC, (core + 1) * B_LOC)
        root_c[sl, 0:128] = out[:, 0:16].T
        root_c[sl, 128:256] = out[:, 16:32].T
        root_c[sl, 256:300] = out[0:44, 32:48].T
        root_h[sl, 0:128] = out[:, 48:64].T
        root_h[sl, 128:256] = out[:, 64:80].T
        root_h[sl, 256:300] = out[0:44, 80:96].T
    _CACHE["last_results"] = res
    return root_c, root_h


# revision 44
# speedup vs baseline: 1.0038x; 1.0038x over previous
"""Trainium2 Bass kernel for a batch-of-trees BinaryTreeLSTM.

Contract: kernel(**inputs) takes the FULL inputs (B=128 trees, 1023-node
complete binary tree, dim 300) and returns the FULL output (root_c, root_h),
each [128, 300] float32.

Strategy
--------
- Data-parallel over trees: 16 trees per NeuronCore x 8 cores, no collectives.
- Mixed precision GEMMs (measured: a DoubleRow matmul costs 1.0 N-cycles and
  covers TWO 128-row K-chunks, i.e. fp8 is 2x fp16 per K-row; only PLAIN fp8
  wins -- residual-compensated fp8 costs exactly fp16):
  * f, i, o gates (sigmoid, contractive): plain fp8 e4m3 DoubleRow, 3 instrs
    per 601-row contraction (kc01, kc34, kc2+zero-slot) vs 5 for fp16.
    Weights x512 (the 1/512 descale rides the ACT's free affine scale);
    child h / leaf x consumed as plain e4m3 (data errors average out).
  * u gate (tanh, slope 1, sensitive): fp16 weights x fp16 h/x.
  Root rel err ~1.3e-2 (gate 2e-2); sigma-gate weight quantization error is
  systematic but tolerable, h8/x8 data error averages out, u stays clean.
- 13 recurrent M-units: [fL0 fL1 fR0 fR1 T9(fL2|fR2) i0 i1 o0 o1 T8(i2|o2) |
  u0 u1 T12(u2)], tails at partition offsets 0/64; 8 leaf units (io fp8,
  u fp16).  sigma units are contiguous so tail blocks (PB<=256) merge all
  of them into one PSUM group / one wide ACT.
- States per level stored E|O-split, written in place by the gate element-
  wise ops: c16/h16 [128, 5, R/2] (slots E0 E1 O0 O1 t2p), h8 [128, 3, 2,
  R/2] (DR pairs (E0,E1),(O0,O1),(t2p, zeros)); h16 feeds only the u GEMM.
- Every block is software-pipelined into A (GEMMs + gate ACTs + c chain) and
  B (tanh-c + h stores), emitted A(n), B(n-1): the per-block cross-engine
  round trip hides under the next block's A.  h8 stores precede h16 so the
  next level's fp8 GEMM starts earliest in the serial deep-tail levels.
- Levels 1 and 2 are cascaded into the leaf-block loop so their state tiles
  are short-lived pools (SBUF would not fit persistent level-1 state).
- Baseline 353.2 us -> 343.8 us; engines: PE ~249 us busy, DVE ~225,
  ACT ~232.  Leaf phase is ACT-bound, recurrent big blocks PE-bound; the
  rest is startup (~18 us) + the inherently serial deep-tail levels (~32 us,
  each a GEMM-issue + ACT + DVE + tanh round trip on <=256 columns).
"""

import os
import sys

for _p in ("/opt/trn_rl_repo",):
    if os.path.isdir(_p) and _p not in sys.path:
        sys.path.insert(0, _p)

import numpy as np
from contextlib import ExitStack

import concourse.bass as bass
import concourse.tile as tile
from concourse import bacc, mybir
from concourse.bass_utils import run_bass_kernel_spmd

# ---------------------------------------------------------------- constants
N_CORES = 8
B = 128
B_LOC = B // N_CORES          # 16 trees per core
N_LEAVES = 512
MEM = 300
XCOLS = N_LEAVES * B_LOC      # 8192 leaf columns per core
LB = 1024                     # leaf-block columns (64 leaves)
NF = 512                      # block moving dim
R_LVL = {l: XCOLS >> l for l in range(1, 10)}

F16 = mybir.dt.float16
F32 = mybir.dt.float32
F8 = mybir.dt.float8e4
AF = mybir.ActivationFunctionType
SIG = AF.Sigmoid
TANH = AF.Tanh
MUL = mybir.AluOpType.mult
ADD = mybir.AluOpType.add
DR = mybir.MatmulPerfMode.DoubleRow
SW = 512.0                    # fp8 weight scale (descaled in ACT)

# fp8 units (f, i, o) -> Wcat column ranges (Wcat = [i o u fL fR] = 1500)
REC8_UNITS = [
    [(0, 128, 900)], [(0, 128, 1028)],       # fL0 fL1
    [(0, 128, 1200)], [(0, 128, 1328)],      # fR0 fR1
    [(0, 44, 1156), (64, 108, 1456)],        # T9 = fL2 | fR2
    [(0, 128, 0)], [(0, 128, 128)],          # i0 i1
    [(0, 128, 300)], [(0, 128, 428)],        # o0 o1
    [(0, 44, 256), (64, 108, 556)],          # T8 = i2 | o2
]
# G gate-column positions (unit index within the 13-wide G tile)
GP_F = 0        # fL01 at 0:2, fR01 at 2:4
GP_T9 = 4
GP_I = 5        # i01 at 5:7
GP_O = 7        # o01 at 7:9
GP_T8 = 9
GP_U = 10       # u01 at 10:12
GP_T12 = 12
# fp16 units (u only)
REC16_UNITS = [
    [(0, 128, 600)], [(0, 128, 728)],        # u0 u1
    [(0, 44, 856)],                          # T12 = u2
]
LEAF8_UNITS = [
    [(0, 128, 0)], [(0, 128, 128)],          # i0 i1
    [(0, 128, 300)], [(0, 128, 428)],        # o0 o1
    [(0, 44, 256), (64, 108, 556)],          # T6 = i2 | o2
]
LEAF16_UNITS = [
    [(0, 128, 600)], [(0, 128, 728)],        # u0 u1
    [(0, 44, 856)],                          # T7 = u2
]


# ---------------------------------------------------------------- host packing
def _q8f(x):
    import ml_dtypes
    return (np.asarray(x, np.float32)
            .astype(ml_dtypes.float8_e4m3fn).astype(np.float32))


def _pack_weights(Wfioux, b_fioux, Wiouh, Wfh):
    """Returns wrec8 [128, 10*3*256] f8, wrec16 [128, 3*5*128] f16,
    wleaf8 [128, 5*2*256] f8, wleaf16 [128, 3*3*128] f16."""
    import ml_dtypes
    f4 = np.float32
    E4 = ml_dtypes.float8_e4m3fn

    Wcat = np.concatenate([Wiouh, Wfh], axis=1).astype(f4)  # [600, 1500]
    bf = np.asarray(b_fioux, f4)
    bias_cat = np.concatenate(
        [bf[300:600], bf[600:900], bf[900:1200], bf[0:300], bf[0:300]])
    A = _q8f(SW * Wcat)
    Ab = _q8f(SW * bias_cat)

    # wrec8: [p, unit(10), pair(3), two(2), m(128)]
    wrec8 = np.zeros((128, 10, 3, 2, 128), f4)
    for u, cols in enumerate(REC8_UNITS):
        for (m0, m1, c0) in cols:
            w = m1 - m0
            wrec8[:, u, 0, 0, m0:m1] = A[0:128, c0:c0 + w]
            wrec8[:, u, 0, 1, m0:m1] = A[128:256, c0:c0 + w]
            wrec8[:, u, 1, 0, m0:m1] = A[300:428, c0:c0 + w]
            wrec8[:, u, 1, 1, m0:m1] = A[428:556, c0:c0 + w]
            wrec8[0:44, u, 2, 0, m0:m1] = A[256:300, c0:c0 + w]
            wrec8[44, u, 2, 0, m0:m1] = Ab[c0:c0 + w]
            wrec8[64:108, u, 2, 0, m0:m1] = A[556:600, c0:c0 + w]
            # pair2 slot1 stays zero (rhs slot is a zero-padded band)

    # wrec16: [p, unit(3), kc(5), m(128)]; kc = (E0, E1, t2p, O0, O1)
    wrec16 = np.zeros((128, 3, 5, 128), f4)
    for u, cols in enumerate(REC16_UNITS):
        for (m0, m1, c0) in cols:
            w = m1 - m0
            wrec16[:, u, 0, m0:m1] = Wcat[0:128, c0:c0 + w]
            wrec16[:, u, 1, m0:m1] = Wcat[128:256, c0:c0 + w]
            wrec16[0:44, u, 2, m0:m1] = Wcat[256:300, c0:c0 + w]
            wrec16[44, u, 2, m0:m1] = bias_cat[c0:c0 + w]
            wrec16[64:108, u, 2, m0:m1] = Wcat[556:600, c0:c0 + w]
            wrec16[:, u, 3, m0:m1] = Wcat[300:428, c0:c0 + w]
            wrec16[:, u, 4, m0:m1] = Wcat[428:556, c0:c0 + w]

    Wl = np.asarray(Wfioux, f4)[:, 300:1200]     # [300, 900]
    bl = bf[300:1200]
    Al = _q8f(SW * Wl)
    Abl = _q8f(SW * bl)

    # wleaf8: [p, unit(5), pair(2), two(2), m(128)]; pair1 = (kc2, zeros)
    wleaf8 = np.zeros((128, 5, 2, 2, 128), f4)
    for u, cols in enumerate(LEAF8_UNITS):
        for (m0, m1, c0) in cols:
            w = m1 - m0
            wleaf8[:, u, 0, 0, m0:m1] = Al[0:128, c0:c0 + w]
            wleaf8[:, u, 0, 1, m0:m1] = Al[128:256, c0:c0 + w]
            wleaf8[0:44, u, 1, 0, m0:m1] = Al[256:300, c0:c0 + w]
            wleaf8[44, u, 1, 0, m0:m1] = Abl[c0:c0 + w]

    # wleaf16: [p, unit(3), kc(3), m(128)]
    wleaf16 = np.zeros((128, 3, 3, 128), f4)
    for u, cols in enumerate(LEAF16_UNITS):
        for (m0, m1, c0) in cols:
            w = m1 - m0
            wleaf16[:, u, 0, m0:m1] = Wl[0:128, c0:c0 + w]
            wleaf16[:, u, 1, m0:m1] = Wl[128:256, c0:c0 + w]
            wleaf16[0:44, u, 2, m0:m1] = Wl[256:300, c0:c0 + w]
            wleaf16[44, u, 2, m0:m1] = bl[c0:c0 + w]

    return (wrec8.reshape(128, -1).astype(E4),
            wrec16.reshape(128, -1).astype(np.float16),
            wleaf8.reshape(128, -1).astype(E4),
            wleaf16.reshape(128, -1).astype(np.float16))


def _check_topology(left_idx, right_idx, leaf_mask):
    li = np.asarray(left_idx); ri = np.asarray(right_idx)
    prev = np.arange(N_LEAVES); nid = N_LEAVES
    ok = bool((np.asarray(leaf_mask)[:N_LEAVES] == 1).all())
    ok &= bool((np.asarray(leaf_mask)[N_LEAVES:] == 0).all())
    while len(prev) > 1:
        cur = []
        for k in range(0, len(prev), 2):
            ok &= bool(li[nid] == prev[k]) and bool(ri[nid] == prev[k + 1])
            cur.append(nid); nid += 1
        prev = np.asarray(cur)
    return ok


# ---------------------------------------------------------------- bass program
def build_program():
    nc = bacc.Bacc("TRN2", target_bir_lowering=False, debug=False)

    xt_d = nc.dram_tensor("xt", [128, 2, XCOLS], F16, kind="ExternalInput").ap()
    x2t_d = nc.dram_tensor("x2t", [44, XCOLS], F16, kind="ExternalInput").ap()
    x8t_d = nc.dram_tensor("x8t", [128, 4, XCOLS], F8,
                           kind="ExternalInput").ap()
    wrec8_d = nc.dram_tensor("wrec8", [128, 10 * 3 * 256], F8,
                             kind="ExternalInput").ap()
    wrec16_d = nc.dram_tensor("wrec16", [128, 3 * 5 * 128], F16,
                              kind="ExternalInput").ap()
    wleaf8_d = nc.dram_tensor("wleaf8", [128, 5 * 2 * 256], F8,
                              kind="ExternalInput").ap()
    wleaf16_d = nc.dram_tensor("wleaf16", [128, 3 * 3 * 128], F16,
                               kind="ExternalInput").ap()
    cons_d = nc.dram_tensor("cons", [84, 2 * LB], F16, kind="ExternalInput").ap()
    cons8_d = nc.dram_tensor("cons8", [84, LB], F8, kind="ExternalInput").ap()
    out_d = nc.dram_tensor("out", [128, 6 * B_LOC], F16,
                           kind="ExternalOutput").ap()

    with ExitStack() as ctx:
        tc = ctx.enter_context(tile.TileContext(nc))
        _build(ctx, tc, xt_d, x2t_d, x8t_d, wrec8_d, wrec16_d, wleaf8_d,
               wleaf16_d, cons_d, cons8_d, out_d)

    nc.compile()
    return nc


def _build(ctx, tc, xt_d, x2t_d, x8t_d, wrec8_d, wrec16_d, wleaf8_d,
           wleaf16_d, cons_d, cons8_d, out_d):
    nc = tc.nc

    wpool = ctx.enter_context(tc.tile_pool(name="wpool", bufs=1))
    state_pool = ctx.enter_context(tc.tile_pool(name="state", bufs=1))

    # ---- weights resident in SBUF (leaf weights first: needed immediately)
    wleaf16_t = wpool.tile([128, 3, 3, 128], F16, name="wleaf16")
    nc.sync.dma_start(wleaf16_t[:], wleaf16_d[:].rearrange(
        "p (u kc m) -> p u kc m", u=3, kc=3))
    wleaf8_t = wpool.tile([128, 5, 2, 2, 128], F8, name="wleaf8")
    nc.sync.dma_start(wleaf8_t[:], wleaf8_d[:].rearrange(
        "p (u pr two m) -> p u pr two m", u=5, pr=2, two=2))
    wrec8_t = wpool.tile([128, 10, 3, 2, 128], F8, name="wrec8")
    wrec16_t = wpool.tile([128, 3, 5, 128], F16, name="wrec16")

    # ---- persistent state for levels 2..8: c16/h16 [128, 5, R/2] slots
    # (E0 E1 O0 O1 t2p), h8 [128, 3, 2, R/2] pairs ((E0,E1),(O0,O1),(t2p,0))
    ST = {}
    for lvl in range(2, 9):
        R = R_LVL[lvl]
        ST[lvl] = dict(
            c=state_pool.tile([128, 5, R // 2], F16, name=f"c_{lvl}"),
            h16=state_pool.tile([128, 5, R // 2], F16, name=f"h16_{lvl}"),
            h8=state_pool.tile([128, 3, 2, R // 2], F8, name=f"h8_{lvl}"),
            R=R)

    # leaf cells (per leaf block, 2 bufs) and L1 cells (1024 L1-cols each)
    LC = [dict(c=state_pool.tile([128, 5, LB // 2], F16, name=f"lc{i}"),
               h16=state_pool.tile([128, 5, LB // 2], F16, name=f"lh16{i}"),
               h8=state_pool.tile([128, 3, 2, LB // 2], F8, name=f"lh8{i}"),
               R=LB) for i in range(2)]
    C1 = [dict(c=state_pool.tile([128, 5, NF], F16, name=f"c1_{i}"),
               h16=state_pool.tile([128, 5, NF], F16, name=f"h16_1{i}"),
               h8=state_pool.tile([128, 3, 2, NF], F8, name=f"h8_1{i}"),
               R=2 * NF) for i in range(2)]

    # persistent x2 (fp16 leaf tail chunk with bias/zero rows)
    x2_t = [state_pool.tile([128, LB], F16, name=f"x2_{i}") for i in range(2)]

    def _pad_cell(cell):
        W = cell["R"] // 2
        nc.sync.dma_start(cell["h16"][44:64, 4, :], cons_d[0:20, :W])
        nc.sync.dma_start(cell["h16"][108:128, 4, :], cons_d[1:21, :W])
        nc.sync.dma_start(cell["c"][44:64, 4, :], cons_d[1:21, :W])
        nc.sync.dma_start(cell["c"][108:128, 4, :], cons_d[1:21, :W])
        nc.sync.dma_start(cell["h8"][44:64, 2, 0, :], cons8_d[0:20, :W])
        nc.sync.dma_start(cell["h8"][108:128, 2, 0, :], cons8_d[1:21, :W])
        # pair-2 slot 1: fully zero (matching weight slot is zero too)
        nc.sync.dma_start(cell["h8"][0:64, 2, 1, :], cons8_d[1:65, :W])
        nc.sync.dma_start(cell["h8"][64:128, 2, 1, :], cons8_d[1:65, :W])

    def _pad_x2():
        for i in range(2):
            nc.sync.dma_start(x2_t[i][44:128, :], cons_d[0:84, :LB])

    def _pad_dmas_early():
        for cell in LC + C1:
            _pad_cell(cell)

    def _pad_dmas_late():
        for lvl in range(2, 9):
            _pad_cell(ST[lvl])

    # ---- pools
    xpool = ctx.enter_context(tc.tile_pool(name="xpool", bufs=2))
    glpool = ctx.enter_context(tc.tile_pool(name="gl", bufs=2))
    gpool = ctx.enter_context(tc.tile_pool(name="g", bufs=2))
    pspool = ctx.enter_context(tc.tile_pool(name="ps", bufs=2, space="PSUM"))
    thpool = ctx.enter_context(tc.tile_pool(name="th", bufs=2))
    tpool = ctx.enter_context(tc.tile_pool(name="t", bufs=2))
    opool = ctx.enter_context(tc.tile_pool(name="o", bufs=1))

    # ================================================================ helpers
    def fp8_group(ps, units, rhs_pairs, PB):
        """Plain-fp8 DoubleRow GEMMs: 3 pair-instrs per unit."""
        for j, u in enumerate(units):
            for k in range(3):
                nc.tensor.matmul(ps[:, j * PB:(j + 1) * PB],
                                 wrec8_t[:, u, k, :, :], rhs_pairs[k],
                                 start=(k == 0), stop=(k == 2),
                                 perf_mode=DR)

    def fp16_group(ps, j0, units, wt, rhs_chunks, PB):
        nkc = len(rhs_chunks)
        for j, u in enumerate(units):
            for k in range(nkc):
                nc.tensor.matmul(ps[:, (j0 + j) * PB:(j0 + j + 1) * PB],
                                 wt[:, u, k, :], rhs_chunks[k],
                                 start=(k == 0), stop=(k == nkc - 1))

    def rec_block(lvl, q0, PB, prev, pq0, dst):
        """One recurrent block: cols q0:q0+PB at level lvl; children at
        E/O positions pq0:pq0+PB of `prev`; dst = cell dict or "root".

        Emits the A-part (GEMMs, gate ACTs, c-chain DVE) inline and returns
        the B-part (tanh-c ACTs + h stores) as a closure, so the caller can
        software-pipeline B behind the next block's A."""
        hw = PB // 2
        h8p = prev["h8"]
        rhs_pairs = [h8p[:, 0, :, pq0:pq0 + PB], h8p[:, 1, :, pq0:pq0 + PB],
                     h8p[:, 2, :, pq0:pq0 + PB]]
        h16p = prev["h16"]
        rhs16 = [h16p[:, 0, pq0:pq0 + PB], h16p[:, 1, pq0:pq0 + PB],
                 h16p[:, 4, pq0:pq0 + PB], h16p[:, 2, pq0:pq0 + PB],
                 h16p[:, 3, pq0:pq0 + PB]]

        G = gpool.tile([128, 13 * NF], F16, tag="G", name=f"G{lvl}")

        # G layout: [fL0 fL1 fR0 fR1 T9 i0 i1 o0 | o1 T8 | u0 u1 T12]
        # (sigma units contiguous at 0:10, tanh at 10:13)
        if PB > 256:
            ps1 = pspool.tile([128, 4 * NF], F32, tag="ps", name="ps1")
            fp8_group(ps1[:, :4 * PB], (0, 1, 2, 3), rhs_pairs, PB)
            ps2 = pspool.tile([128, 4 * NF], F32, tag="ps", name="ps2")
            fp8_group(ps2[:, :4 * PB], (4, 5, 6, 7), rhs_pairs, PB)
            nc.scalar.activation(G[:, 0:4 * PB], ps1[:, :4 * PB], SIG,
                                 scale=1.0 / SW)
            ps3 = pspool.tile([128, 4 * NF], F32, tag="ps", name="ps3")
            fp8_group(ps3[:, :2 * PB], (8, 9), rhs_pairs, PB)
            fp16_group(ps3, 2, (0, 1), wrec16_t, rhs16, PB)
            nc.scalar.activation(G[:, 4 * PB:8 * PB], ps2[:, :4 * PB], SIG,
                                 scale=1.0 / SW)
            ps4 = pspool.tile([128, 4 * NF], F32, tag="ps", name="ps4")
            fp16_group(ps4, 0, (2,), wrec16_t, rhs16, PB)
            nc.scalar.activation(G[:, 8 * PB:10 * PB], ps3[:, 0:2 * PB], SIG,
                                 scale=1.0 / SW)
            nc.scalar.activation(G[:, 10 * PB:12 * PB], ps3[:, 2 * PB:4 * PB],
                                 TANH)
            nc.scalar.activation(G[:, 12 * PB:13 * PB], ps4[:, 0:PB], TANH)
        else:
            # tail blocks: merged groups, fewer ACTs / PSUM round-trips
            n8 = 8 if PB == 256 else 10
            ps1 = pspool.tile([128, 4 * NF], F32, tag="ps", name="ps1")
            fp8_group(ps1[:, :n8 * PB], tuple(range(n8)), rhs_pairs, PB)
            ps2 = pspool.tile([128, 4 * NF], F32, tag="ps", name="ps2")
            j0 = 0
            if n8 == 8:
                fp8_group(ps2[:, :2 * PB], (8, 9), rhs_pairs, PB)
                j0 = 2
            fp16_group(ps2, j0, (0, 1, 2), wrec16_t, rhs16, PB)
            nc.scalar.activation(G[:, 0:n8 * PB], ps1[:, :n8 * PB], SIG,
                                 scale=1.0 / SW)
            if n8 == 8:
                nc.scalar.activation(G[:, 8 * PB:10 * PB], ps2[:, 0:2 * PB],
                                     SIG, scale=1.0 / SW)
            nc.scalar.activation(G[:, 10 * PB:13 * PB],
                                 ps2[:, j0 * PB:(j0 + 3) * PB], TANH)

        # ---- elementwise
        cp = prev["c"]
        c2p = cp[:, 4, pq0:pq0 + PB]           # [p, PB] (E@0:44, O@64:108)

        t1 = tpool.tile([128, 4, NF], F16, tag="t1", name="t1", bufs=1)
        t12a = tpool.tile([64, NF], F16, tag="t12a", name="t12a", bufs=1)
        t12b = tpool.tile([64, NF], F16, tag="t12b", name="t12b", bufs=1)
        fc = tpool.tile([128, 2, NF], F16, tag="fc", name="fc", bufs=1)
        fc2 = tpool.tile([64, NF], F16, tag="fc2", name="fc2", bufs=1)
        iu = tpool.tile([128, 2, NF], F16, tag="iu", name="iu", bufs=1)
        iu2 = tpool.tile([64, NF], F16, tag="iu2", name="iu2", bufs=1)

        g2 = G[:, 0:13 * PB].rearrange("p (u n) -> p u n", u=13)
        nc.vector.tensor_tensor(t1[:, :, :PB], g2[:, 0:4, :],
                                cp[:, 0:4, pq0:pq0 + PB], MUL)
        nc.vector.tensor_tensor(t12a[0:44, :PB], g2[0:44, GP_T9, :],
                                c2p[0:44], MUL)
        nc.vector.tensor_tensor(t12b[0:44, :PB], g2[64:108, GP_T9, :],
                                c2p[64:108], MUL)
        nc.vector.tensor_tensor(fc[:, :, :PB], t1[:, 0:2, :PB],
                                t1[:, 2:4, :PB], ADD)
        nc.vector.tensor_tensor(fc2[0:44, :PB], t12a[0:44, :PB],
                                t12b[0:44, :PB], ADD)
        nc.vector.tensor_tensor(iu[:, :, :PB], g2[:, GP_I:GP_I + 2, :],
                                g2[:, GP_U:GP_U + 2, :], MUL)
        nc.vector.tensor_tensor(iu2[0:44, :PB], g2[0:44, GP_T8, :],
                                g2[0:44, GP_T12, :], MUL)

        if dst == "root":
            ot = opool.tile([128, 6 * B_LOC], F16, name="ot")
            nc.sync.dma_start(ot[44:128, 2 * B_LOC:3 * B_LOC],
                              cons_d[0:84, B_LOC:2 * B_LOC])
            nc.sync.dma_start(ot[44:128, 5 * B_LOC:6 * B_LOC],
                              cons_d[0:84, B_LOC:2 * B_LOC])
            oc = ot[:, 0:2 * B_LOC].rearrange("p (c n) -> p c n", c=2)
            nc.vector.tensor_tensor(oc, iu[:, :, :PB], fc[:, :, :PB], ADD)
            nc.vector.tensor_tensor(ot[0:44, 2 * B_LOC:3 * B_LOC],
                                    iu2[0:44, :PB], fc2[0:44, :PB], ADD)
            tho = thpool.tile([128, 5, NF // 2], F16, tag="th", name="tho")
            nc.scalar.activation(tho[:, 0:2, :PB], oc, TANH)
            nc.scalar.activation(tho[64:108, 2, :PB],
                                 ot[0:44, 2 * B_LOC:3 * B_LOC], TANH)
            oh = ot[:, 3 * B_LOC:5 * B_LOC].rearrange("p (c n) -> p c n", c=2)
            nc.vector.tensor_tensor(oh, g2[:, GP_O:GP_O + 2, :],
                                    tho[:, 0:2, :PB], MUL)
            nc.vector.tensor_tensor(ot[0:44, 5 * B_LOC:6 * B_LOC],
                                    g2[64:108, GP_T8, :],
                                    tho[64:108, 2, :PB], MUL)
            nc.sync.dma_start(out_d[:, :], ot[:, :])
            return None

        qh = q0 // 2
        ct, h16t, h8t = dst["c"], dst["h16"], dst["h8"]
        # c store, E|O split (two ops: ISA allows at most 3 free dims)
        iu4 = iu[:, :, :PB].rearrange("p ch (m two b) -> p two ch m b",
                                      two=2, b=B_LOC)
        fc4 = fc[:, :, :PB].rearrange("p ch (m two b) -> p two ch m b",
                                      two=2, b=B_LOC)
        ce = ct[:, 0:2, qh:qh + hw].rearrange("p c (m b) -> p c m b", b=B_LOC)
        cod = ct[:, 2:4, qh:qh + hw].rearrange("p c (m b) -> p c m b", b=B_LOC)
        nc.vector.tensor_tensor(ce, iu4[:, 0], fc4[:, 0], ADD)
        nc.vector.tensor_tensor(cod, iu4[:, 1], fc4[:, 1], ADD)
        # c tail: interleaved at band [64:108], then E|O copies into the state
        tci = tpool.tile([128, NF], F16, tag="tci", name="tci")
        nc.vector.tensor_tensor(tci[64:108, :PB], iu2[0:44, :PB],
                                fc2[0:44, :PB], ADD)
        tcv = tci[64:108, :PB].rearrange("p (m two b) -> p m two b",
                                         two=2, b=B_LOC)
        c2o = ct[:, 4, qh:qh + hw].rearrange("p (m b) -> p m b", b=B_LOC)
        nc.vector.tensor_scalar_mul(c2o[0:44], tcv[:, :, 0, :], 1.0)
        nc.vector.tensor_scalar_mul(c2o[64:108], tcv[:, :, 1, :], 1.0)

        def finish():
            # tanh: main slots from the state, tail from the interleaved band
            th = thpool.tile([128, 5, NF // 2], F16, tag="th", name="th")
            nc.scalar.activation(th[:, 0:4, :hw], ct[:, 0:4, qh:qh + hw],
                                 TANH)
            th2i = tpool.tile([128, NF], F16, tag="th2i", name="th2i")
            nc.scalar.activation(th2i[64:108, :PB], tci[64:108, :PB], TANH)

            # h stores: h16 (GpSimd, off critical path) and h8 (DVE)
            o4 = g2[:, GP_O:GP_O + 2, :].rearrange(
                "p ch (m two b) -> p two ch m b", two=2, b=B_LOC)
            o2g = g2[64:108, GP_T8, :].rearrange("p (m two b) -> p m two b",
                                                 two=2, b=B_LOC)
            thr = th[:, :, :hw]
            the = thr[:, 0:2, :].rearrange("p ch (m b) -> p ch m b", b=B_LOC)
            tho_ = thr[:, 2:4, :].rearrange("p ch (m b) -> p ch m b", b=B_LOC)
            th2 = th2i[64:108, :PB].rearrange("p (m two b) -> p m two b",
                                              two=2, b=B_LOC)

            h16e = h16t[:, 0:2, qh:qh + hw].rearrange("p c (m b) -> p c m b",
                                                      b=B_LOC)
            h16o = h16t[:, 2:4, qh:qh + hw].rearrange("p c (m b) -> p c m b",
                                                      b=B_LOC)
            h16_2 = h16t[:, 4, qh:qh + hw].rearrange("p (m b) -> p m b",
                                                     b=B_LOC)
            nc.vector.tensor_tensor(h16e, o4[:, 0], the, MUL)
            nc.vector.tensor_tensor(h16o, o4[:, 1], tho_, MUL)
            nc.vector.tensor_tensor(h16_2[0:44], o2g[:, :, 0, :],
                                    th2[:, :, 0, :], MUL)
            nc.vector.tensor_tensor(h16_2[64:108], o2g[:, :, 1, :],
                                    th2[:, :, 1, :], MUL)

            h8e = h8t[:, 0, :, qh:qh + hw].rearrange("p c (m b) -> p c m b",
                                                     b=B_LOC)
            h8o = h8t[:, 1, :, qh:qh + hw].rearrange("p c (m b) -> p c m b",
                                                     b=B_LOC)
            nc.vector.tensor_tensor(h8e, o4[:, 0], the, MUL)
            nc.vector.tensor_tensor(h8o, o4[:, 1], tho_, MUL)
            h8_2 = h8t[:, 2, 0, qh:qh + hw].rearrange("p (m b) -> p m b",
                                                      b=B_LOC)
            nc.vector.tensor_tensor(h8_2[0:44], o2g[:, :, 0, :],
                                    th2[:, :, 0, :], MUL)
            nc.vector.tensor_tensor(h8_2[64:108], o2g[:, :, 1, :],
                                    th2[:, :, 1, :], MUL)

        return finish

    # ---------------------------------------------------------------- leaves
    def leaf_sub(x01, x2, x8, s, cell):
        """Leaf sub-chunk (512 cols): GEMM + gate ACTs + c-chain (A-part);
        returns the B-part closure.  Gl layout: [i0 i1 o0 o1 | T6 u0 u1 T7]"""
        n0 = s * NF
        hw = NF // 2
        qh = s * hw
        rhs16 = [x01[:, 0, n0:n0 + NF], x01[:, 1, n0:n0 + NF],
                 x2[:, n0:n0 + NF]]
        x8p = [x8[:, 0, :, n0:n0 + NF], x8[:, 1, :, n0:n0 + NF]]
        Gl = glpool.tile([128, 8 * NF], F16, tag="Gl", name="Gl")
        psA = pspool.tile([128, 4 * NF], F32, tag="ps", name="lpsA")
        for j, u in enumerate((0, 1, 2, 3)):
            for k in range(2):
                nc.tensor.matmul(psA[:, j * NF:(j + 1) * NF],
                                 wleaf8_t[:, u, k, :, :], x8p[k],
                                 start=(k == 0), stop=(k == 1), perf_mode=DR)
        psB = pspool.tile([128, 4 * NF], F32, tag="ps", name="lpsB")
        for k in range(2):
            nc.tensor.matmul(psB[:, 0:NF], wleaf8_t[:, 4, k, :, :], x8p[k],
                             start=(k == 0), stop=(k == 1), perf_mode=DR)
        fp16_group(psB, 1, (0, 1, 2), wleaf16_t, rhs16, NF)
        nc.scalar.activation(Gl[:, 0:4 * NF], psA[:, :], SIG, scale=1.0 / SW)
        nc.scalar.activation(Gl[:, 4 * NF:5 * NF], psB[:, 0:NF], SIG,
                             scale=1.0 / SW)
        nc.scalar.activation(Gl[:, 5 * NF:8 * NF], psB[:, NF:4 * NF], TANH)

        ct, h16t, h8t = cell["c"], cell["h16"], cell["h8"]
        g2 = Gl.rearrange("p (u n) -> p u n", u=8)
        i4 = g2[:, 0:2, :].rearrange("p ch (m two b) -> p two ch m b",
                                     two=2, b=B_LOC)
        u4 = g2[:, 5:7, :].rearrange("p ch (m two b) -> p two ch m b",
                                     two=2, b=B_LOC)

        ce = ct[:, 0:2, qh:qh + hw].rearrange("p c (m b) -> p c m b", b=B_LOC)
        co = ct[:, 2:4, qh:qh + hw].rearrange("p c (m b) -> p c m b", b=B_LOC)
        nc.vector.tensor_tensor(ce, i4[:, 0], u4[:, 0], MUL)
        nc.vector.tensor_tensor(co, i4[:, 1], u4[:, 1], MUL)
        # interleaved tail c at band [64:108], then E|O copies into the state
        tci = tpool.tile([128, NF], F16, tag="tci", name="ltci")
        nc.vector.tensor_tensor(tci[64:108, :NF], g2[0:44, 4, :],
                                g2[0:44, 7, :], MUL)
        tcv = tci[64:108, :NF].rearrange("p (m two b) -> p m two b",
                                         two=2, b=B_LOC)
        c2 = ct[:, 4, qh:qh + hw].rearrange("p (m b) -> p m b", b=B_LOC)
        nc.vector.tensor_scalar_mul(c2[0:44], tcv[:, :, 0, :], 1.0)
        nc.vector.tensor_scalar_mul(c2[64:108], tcv[:, :, 1, :], 1.0)

        def finish():
            o4 = g2[:, 2:4, :].rearrange("p ch (m two b) -> p two ch m b",
                                         two=2, b=B_LOC)
            o2g = g2[64:108, 4, :].rearrange("p (m two b) -> p m two b",
                                             two=2, b=B_LOC)
            th = thpool.tile([128, 5, NF // 2], F16, tag="th", name="lth")
            nc.scalar.activation(th[:, 0:4, :hw], ct[:, 0:4, qh:qh + hw],
                                 TANH)
            th2i = tpool.tile([128, NF], F16, tag="th2i", name="lth2i")
            nc.scalar.activation(th2i[64:108, :NF], tci[64:108, :NF], TANH)
            the = th[:, 0:2, :hw].rearrange("p c (m b) -> p c m b", b=B_LOC)
            tho_ = th[:, 2:4, :hw].rearrange("p c (m b) -> p c m b", b=B_LOC)
            th2 = th2i[64:108, :NF].rearrange("p (m two b) -> p m two b",
                                              two=2, b=B_LOC)

            h16e = h16t[:, 0:2, qh:qh + hw].rearrange("p c (m b) -> p c m b",
                                                      b=B_LOC)
            h16o = h16t[:, 2:4, qh:qh + hw].rearrange("p c (m b) -> p c m b",
                                                      b=B_LOC)
            h16_2 = h16t[:, 4, qh:qh + hw].rearrange("p (m b) -> p m b",
                                                     b=B_LOC)
            nc.vector.tensor_tensor(h16e, o4[:, 0], the, MUL)
            nc.vector.tensor_tensor(h16o, o4[:, 1], tho_, MUL)
            nc.vector.tensor_tensor(h16_2[0:44], o2g[:, :, 0, :],
                                    th2[:, :, 0, :], MUL)
            nc.vector.tensor_tensor(h16_2[64:108], o2g[:, :, 1, :],
                                    th2[:, :, 1, :], MUL)
            h8e = h8t[:, 0, :, qh:qh + hw].rearrange("p c (m b) -> p c m b",
                                                     b=B_LOC)
            h8o = h8t[:, 1, :, qh:qh + hw].rearrange("p c (m b) -> p c m b",
                                                     b=B_LOC)
            nc.vector.tensor_tensor(h8e, o4[:, 0], the, MUL)
            nc.vector.tensor_tensor(h8o, o4[:, 1], tho_, MUL)
            h8_2 = h8t[:, 2, 0, qh:qh + hw].rearrange("p (m b) -> p m b",
                                                      b=B_LOC)
            nc.vector.tensor_tensor(h8_2[0:44], o2g[:, :, 0, :],
                                    th2[:, :, 0, :], MUL)
            nc.vector.tensor_tensor(h8_2[64:108], o2g[:, :, 1, :],
                                    th2[:, :, 1, :], MUL)

        return finish

    # ================================================================ schedule
    # Software pipeline: emit A(block n) then B(block n-1); B(X) must precede
    # A(any consumer of X) -- guaranteed by the global block order below plus
    # explicit flushes at phase-B level boundaries.
    PEND = [None]

    def sched(b_fn):
        if PEND[0] is not None:
            PEND[0]()
        PEND[0] = b_fn

    def flush():
        if PEND[0] is not None:
            PEND[0]()
            PEND[0] = None

    n_blk = XCOLS // LB                       # 8 leaf blocks
    xtiles = {}

    def dma_x(j):
        c0 = j * LB
        x01 = xpool.tile([128, 2, LB], F16, tag="x01", name="x01")
        x8 = xpool.tile([128, 2, 2, LB], F8, tag="x8", name="x8")
        x2 = x2_t[j % 2]
        h = LB // 2
        for t in range(2):
            nc.sync.dma_start(x01[:, :, t * h:(t + 1) * h],
                              xt_d[:, :, c0 + t * h:c0 + (t + 1) * h])
            nc.sync.dma_start(
                x8[:, :, :, t * h:(t + 1) * h].rearrange(
                    "p pr two n -> p (pr two) n"),
                x8t_d[:, :, c0 + t * h:c0 + (t + 1) * h])
            nc.sync.dma_start(x2[0:44, t * h:(t + 1) * h],
                              x2t_d[:, c0 + t * h:c0 + (t + 1) * h])
        xtiles[j] = (x01, x2, x8)

    _pad_x2()
    dma_x(0)
    for blk in range(n_blk):
        x01, x2, x8 = xtiles.pop(blk)
        if blk + 1 < n_blk:
            dma_x(blk + 1)                    # prefetch next block's x
        if blk == 0:
            _pad_dmas_early()
            nc.sync.dma_start(wrec8_t[:], wrec8_d[:].rearrange(
                "p (u pr two m) -> p u pr two m", u=10, pr=3, two=2))
            nc.sync.dma_start(wrec16_t[:], wrec16_d[:].rearrange(
                "p (u kc m) -> p u kc m", u=3, kc=5))
        elif blk == 2:
            _pad_dmas_late()

        cell = LC[blk % 2]
        for s in range(2):
            sched(leaf_sub(x01, x2, x8, s, cell))
        if blk >= 1:
            j = blk - 1
            # L1 block j: output into C1 cell j//2 at col offset (j%2)*NF
            sched(rec_block(1, (j % 2) * NF, NF, LC[j % 2], 0,
                            C1[(j // 2) % 2]))
        if blk >= 3 and blk % 2 == 1:
            k = (blk - 3) // 2
            sched(rec_block(2, k * NF, NF, C1[k % 2], 0, ST[2]))
    # drain: l1(7), l2(3) -- l2(3) consumes l1(7) so flush in between
    sched(rec_block(1, NF, NF, LC[1], 0, C1[1]))
    flush()
    sched(rec_block(2, 3 * NF, NF, C1[1], 0, ST[2]))
    flush()

    # ---------------------------------------------------------------- phase B
    for lvl in range(3, 10):
        R = R_LVL[lvl]
        PB = min(NF, R)
        prev = ST[lvl - 1]
        for q0 in range(0, R, PB):
            b = rec_block(lvl, q0, PB, prev, q0,
                          ST[lvl] if lvl < 9 else "root")
            if b is not None:
                sched(b)
        flush()


# ---------------------------------------------------------------- runner
_CACHE = {}


def _get_program():
    if "nc" not in _CACHE:
        _CACHE["nc"] = build_program()
    return _CACHE["nc"]


def _host_inputs(inputs, Wfioux, b_fioux, Wiouh, Wfh):
    import ml_dtypes
    E4 = ml_dtypes.float8_e4m3fn
    wrec8, wrec16, wleaf8, wleaf16 = _pack_weights(
        np.asarray(Wfioux, np.float32), np.asarray(b_fioux, np.float32),
        np.asarray(Wiouh, np.float32), np.asarray(Wfh, np.float32))
    cons = np.zeros((84, 2 * LB), np.float16)
    cons[0, :] = 1.0
    cons8 = np.zeros((84, LB), np.float32)
    cons8[0, :] = 1.0
    cons8 = cons8.astype(E4)
    in_maps = []
    for core in range(N_CORES):
        x = np.asarray(inputs[core * B_LOC:(core + 1) * B_LOC, :N_LEAVES, :],
                       np.float32)
        xt_full = x.transpose(2, 1, 0).reshape(MEM, XCOLS)
        xt = np.ascontiguousarray(
            xt_full[0:256].reshape(2, 128, XCOLS).transpose(1, 0, 2)
        ).astype(np.float16)
        x2t = np.ascontiguousarray(xt_full[256:300]).astype(np.float16)
        x8t = np.zeros((128, 4, XCOLS), np.float32)
        x8t[:, 0, :] = xt_full[0:128]
        x8t[:, 1, :] = xt_full[128:256]
        x8t[0:44, 2, :] = xt_full[256:300]
        x8t[44, 2, :] = 1.0
        x8t = x8t.astype(E4)
        in_maps.append({"xt": xt, "x2t": x2t, "x8t": x8t, "wrec8": wrec8,
                        "wrec16": wrec16, "wleaf8": wleaf8,
                        "wleaf16": wleaf16, "cons": cons, "cons8": cons8})
    return in_maps


def kernel(inputs, Wfioux, b_fioux, Wiouh, Wfh, left_idx, right_idx, leaf_mask,
           _trace=False, _trace_dir=None):
    inputs = np.asarray(inputs, np.float32)
    assert _check_topology(left_idx, right_idx, leaf_mask), \
        "tree topology does not match the expected complete binary tree"

    in_maps = _host_inputs(inputs, Wfioux, b_fioux, Wiouh, Wfh)
    nc = _get_program()
    res = run_bass_kernel_spmd(nc, in_maps, list(range(N_CORES)),
                               trace=_trace, tmpdir=_trace_dir)

    root_c = np.zeros((B, MEM), np.float32)
    root_h = np.zeros((B, MEM), np.float32)
    for core in range(N_CORES):
        out = np.asarray(res.results[core]["out"], np.float32)  # [128, 96]
        sl = slice(core * B_LOC, (core + 1) * B_LOC)
        root_c[sl, 0:128] = out[:, 0:16].T
        root_c[sl, 128:256] = out[:, 16:32].T
        root_c[sl, 256:300] = out[0:44, 32:48].T
        root_h[sl, 0:128] = out[:, 48:64].T
        root_h[sl, 128:256] = out[:, 64:80].T
        root_h[sl, 256:300] = out[0:44, 80:96].T
    _CACHE["last_results"] = res
    return root_c, root_h


# revision 45
# speedup vs baseline: 1.0271x; 1.0232x over previous
"""Trainium2 Bass kernel for a batch-of-trees BinaryTreeLSTM.

Contract: kernel(**inputs) takes the FULL inputs (B=128 trees, 1023-node
complete binary tree, dim 300) and returns the FULL output (root_c, root_h),
each [128, 300] float32.

Strategy
--------
- Data-parallel over trees: 16 trees per NeuronCore x 8 cores, no collectives.
- Mixed precision GEMMs (measured: a DoubleRow matmul costs 1.0 N-cycles and
  covers TWO 128-row K-chunks, i.e. fp8 is 2x fp16 per K-row; only PLAIN fp8
  wins -- residual-compensated fp8 costs exactly fp16):
  * f, i, o gates (sigmoid, contractive): plain fp8 e4m3 DoubleRow, 3 instrs
    per 601-row contraction (kc01, kc34, kc2+zero-slot) vs 5 for fp16.
    Weights x512 (the 1/512 descale rides the ACT's free affine scale);
    child h / leaf x consumed as plain e4m3 (data errors average out).
  * u gate (tanh, slope 1, sensitive): fp16 weights x fp16 h/x.
  Root rel err ~1.3e-2 (gate 2e-2); sigma-gate weight quantization error is
  systematic but tolerable, h8/x8 data error averages out, u stays clean.
- 13 recurrent M-units: [fL0 fL1 fR0 fR1 T9(fL2|fR2) i0 i1 o0 o1 T8(i2|o2) |
  u0 u1 T12(u2)], tails at partition offsets 0/64; 8 leaf units (io fp8,
  u fp16).  sigma units are contiguous so tail blocks (PB<=256) merge all
  of them into one PSUM group / one wide ACT.
- States per level stored E|O-split, written in place by the gate element-
  wise ops: c16/h16 [128, 5, R/2] (slots E0 E1 O0 O1 t2p), h8 [128, 3, 2,
  R/2] (DR pairs (E0,E1),(O0,O1),(t2p, zeros)); h16 feeds only the u GEMM.
- Every block is software-pipelined into A (GEMMs + gate ACTs + c chain) and
  B (tanh-c + h stores), emitted A(n), B(n-1): the per-block cross-engine
  round trip hides under the next block's A.  h8 stores precede h16 so the
  next level's fp8 GEMM starts earliest in the serial deep-tail levels.
- Levels 1 and 2 are cascaded into the leaf-block loop so their state tiles
  are short-lived pools (SBUF would not fit persistent level-1 state).
- Baseline 353.2 us -> 343.8 us; engines: PE ~249 us busy, DVE ~225,
  ACT ~232.  Leaf phase is ACT-bound, recurrent big blocks PE-bound; the
  rest is startup (~18 us) + the inherently serial deep-tail levels (~32 us,
  each a GEMM-issue + ACT + DVE + tanh round trip on <=256 columns).
"""

import os
import sys

for _p in ("/opt/trn_rl_repo",):
    if os.path.isdir(_p) and _p not in sys.path:
        sys.path.insert(0, _p)

import numpy as np
from contextlib import ExitStack

import concourse.bass as bass
import concourse.tile as tile
from concourse import bacc, mybir
from concourse.bass_utils import run_bass_kernel_spmd

# ---------------------------------------------------------------- constants
N_CORES = 8
B = 128
B_LOC = B // N_CORES          # 16 trees per core
N_LEAVES = 512
MEM = 300
XCOLS = N_LEAVES * B_LOC      # 8192 leaf columns per core
LB = 1024                     # leaf-block columns (64 leaves)
NF = 512                      # block moving dim
R_LVL = {l: XCOLS >> l for l in range(1, 10)}

F16 = mybir.dt.float16
F32 = mybir.dt.float32
F8 = mybir.dt.float8e4
AF = mybir.ActivationFunctionType
SIG = AF.Sigmoid
TANH = AF.Tanh
MUL = mybir.AluOpType.mult
ADD = mybir.AluOpType.add
DR = mybir.MatmulPerfMode.DoubleRow
SW = 512.0                    # fp8 weight scale (descaled in ACT)

# fp8 units (f, i, o) -> Wcat column ranges (Wcat = [i o u fL fR] = 1500)
REC8_UNITS = [
    [(0, 128, 900)], [(0, 128, 1028)],       # fL0 fL1
    [(0, 128, 1200)], [(0, 128, 1328)],      # fR0 fR1
    [(0, 44, 1156), (64, 108, 1456)],        # T9 = fL2 | fR2
    [(0, 128, 0)], [(0, 128, 128)],          # i0 i1
    [(0, 128, 300)], [(0, 128, 428)],        # o0 o1
    [(0, 44, 256), (64, 108, 556)],          # T8 = i2 | o2
]
# G gate-column positions (unit index within the 13-wide G tile)
GP_F = 0        # fL01 at 0:2, fR01 at 2:4
GP_T9 = 4
GP_I = 5        # i01 at 5:7
GP_O = 7        # o01 at 7:9
GP_T8 = 9
GP_U = 10       # u01 at 10:12
GP_T12 = 12
# fp16 units (u only)
REC16_UNITS = [
    [(0, 128, 600)], [(0, 128, 728)],        # u0 u1
    [(0, 44, 856)],                          # T12 = u2
]
LEAF8_UNITS = [
    [(0, 128, 0)], [(0, 128, 128)],          # i0 i1
    [(0, 128, 300)], [(0, 128, 428)],        # o0 o1
    [(0, 44, 256), (64, 108, 556)],          # T6 = i2 | o2
]
LEAF16_UNITS = [
    [(0, 128, 600)], [(0, 128, 728)],        # u0 u1
    [(0, 44, 856)],                          # T7 = u2
]


# ---------------------------------------------------------------- host packing
def _q8f(x):
    import ml_dtypes
    return (np.asarray(x, np.float32)
            .astype(ml_dtypes.float8_e4m3fn).astype(np.float32))


def _pack_weights(Wfioux, b_fioux, Wiouh, Wfh):
    """Returns wrec8 [128, 10*3*256] f8, wrec16 [128, 3*5*128] f16,
    wleaf8 [128, 5*2*256] f8, wleaf16 [128, 3*3*128] f16."""
    import ml_dtypes
    f4 = np.float32
    E4 = ml_dtypes.float8_e4m3fn

    Wcat = np.concatenate([Wiouh, Wfh], axis=1).astype(f4)  # [600, 1500]
    bf = np.asarray(b_fioux, f4)
    bias_cat = np.concatenate(
        [bf[300:600], bf[600:900], bf[900:1200], bf[0:300], bf[0:300]])
    A = _q8f(SW * Wcat)
    Ab = _q8f(SW * bias_cat)

    # wrec8: [p, unit(10), pair(3), two(2), m(128)]
    wrec8 = np.zeros((128, 10, 3, 2, 128), f4)
    for u, cols in enumerate(REC8_UNITS):
        for (m0, m1, c0) in cols:
            w = m1 - m0
            wrec8[:, u, 0, 0, m0:m1] = A[0:128, c0:c0 + w]
            wrec8[:, u, 0, 1, m0:m1] = A[128:256, c0:c0 + w]
            wrec8[:, u, 1, 0, m0:m1] = A[300:428, c0:c0 + w]
            wrec8[:, u, 1, 1, m0:m1] = A[428:556, c0:c0 + w]
            wrec8[0:44, u, 2, 0, m0:m1] = A[256:300, c0:c0 + w]
            wrec8[44, u, 2, 0, m0:m1] = Ab[c0:c0 + w]
            wrec8[64:108, u, 2, 0, m0:m1] = A[556:600, c0:c0 + w]
            # pair2 slot1 stays zero (rhs slot is a zero-padded band)

    # wrec16: [p, unit(3), kc(5), m(128)]; kc = (E0, E1, t2p, O0, O1)
    wrec16 = np.zeros((128, 3, 5, 128), f4)
    for u, cols in enumerate(REC16_UNITS):
        for (m0, m1, c0) in cols:
            w = m1 - m0
            wrec16[:, u, 0, m0:m1] = Wcat[0:128, c0:c0 + w]
            wrec16[:, u, 1, m0:m1] = Wcat[128:256, c0:c0 + w]
            wrec16[0:44, u, 2, m0:m1] = Wcat[256:300, c0:c0 + w]
            wrec16[44, u, 2, m0:m1] = bias_cat[c0:c0 + w]
            wrec16[64:108, u, 2, m0:m1] = Wcat[556:600, c0:c0 + w]
            wrec16[:, u, 3, m0:m1] = Wcat[300:428, c0:c0 + w]
            wrec16[:, u, 4, m0:m1] = Wcat[428:556, c0:c0 + w]

    Wl = np.asarray(Wfioux, f4)[:, 300:1200]     # [300, 900]
    bl = bf[300:1200]
    Al = _q8f(SW * Wl)
    Abl = _q8f(SW * bl)

    # wleaf8: [p, unit(5), pair(2), two(2), m(128)]; pair1 = (kc2, zeros)
    wleaf8 = np.zeros((128, 5, 2, 2, 128), f4)
    for u, cols in enumerate(LEAF8_UNITS):
        for (m0, m1, c0) in cols:
            w = m1 - m0
            wleaf8[:, u, 0, 0, m0:m1] = Al[0:128, c0:c0 + w]
            wleaf8[:, u, 0, 1, m0:m1] = Al[128:256, c0:c0 + w]
            wleaf8[0:44, u, 1, 0, m0:m1] = Al[256:300, c0:c0 + w]
            wleaf8[44, u, 1, 0, m0:m1] = Abl[c0:c0 + w]

    # wleaf16: [p, unit(3), kc(3), m(128)]
    wleaf16 = np.zeros((128, 3, 3, 128), f4)
    for u, cols in enumerate(LEAF16_UNITS):
        for (m0, m1, c0) in cols:
            w = m1 - m0
            wleaf16[:, u, 0, m0:m1] = Wl[0:128, c0:c0 + w]
            wleaf16[:, u, 1, m0:m1] = Wl[128:256, c0:c0 + w]
            wleaf16[0:44, u, 2, m0:m1] = Wl[256:300, c0:c0 + w]
            wleaf16[44, u, 2, m0:m1] = bl[c0:c0 + w]

    return (wrec8.reshape(128, -1).astype(E4),
            wrec16.reshape(128, -1).astype(np.float16),
            wleaf8.reshape(128, -1).astype(E4),
            wleaf16.reshape(128, -1).astype(np.float16))


def _check_topology(left_idx, right_idx, leaf_mask):
    li = np.asarray(left_idx); ri = np.asarray(right_idx)
    prev = np.arange(N_LEAVES); nid = N_LEAVES
    ok = bool((np.asarray(leaf_mask)[:N_LEAVES] == 1).all())
    ok &= bool((np.asarray(leaf_mask)[N_LEAVES:] == 0).all())
    while len(prev) > 1:
        cur = []
        for k in range(0, len(prev), 2):
            ok &= bool(li[nid] == prev[k]) and bool(ri[nid] == prev[k + 1])
            cur.append(nid); nid += 1
        prev = np.asarray(cur)
    return ok


# ---------------------------------------------------------------- bass program
def build_program():
    nc = bacc.Bacc("TRN2", target_bir_lowering=False, debug=False)

    xt_d = nc.dram_tensor("xt", [128, 2, XCOLS], F16, kind="ExternalInput").ap()
    x2t_d = nc.dram_tensor("x2t", [44, XCOLS], F16, kind="ExternalInput").ap()
    x8t_d = nc.dram_tensor("x8t", [128, 4, XCOLS], F8,
                           kind="ExternalInput").ap()
    wrec8_d = nc.dram_tensor("wrec8", [128, 10 * 3 * 256], F8,
                             kind="ExternalInput").ap()
    wrec16_d = nc.dram_tensor("wrec16", [128, 3 * 5 * 128], F16,
                              kind="ExternalInput").ap()
    wleaf8_d = nc.dram_tensor("wleaf8", [128, 5 * 2 * 256], F8,
                              kind="ExternalInput").ap()
    wleaf16_d = nc.dram_tensor("wleaf16", [128, 3 * 3 * 128], F16,
                               kind="ExternalInput").ap()
    cons_d = nc.dram_tensor("cons", [84, 2 * LB], F16, kind="ExternalInput").ap()
    cons8_d = nc.dram_tensor("cons8", [84, LB], F8, kind="ExternalInput").ap()
    out_d = nc.dram_tensor("out", [128, 6 * B_LOC], F16,
                           kind="ExternalOutput").ap()

    with ExitStack() as ctx:
        tc = ctx.enter_context(tile.TileContext(nc))
        _build(ctx, tc, xt_d, x2t_d, x8t_d, wrec8_d, wrec16_d, wleaf8_d,
               wleaf16_d, cons_d, cons8_d, out_d)

    nc.compile()
    return nc


def _build(ctx, tc, xt_d, x2t_d, x8t_d, wrec8_d, wrec16_d, wleaf8_d,
           wleaf16_d, cons_d, cons8_d, out_d):
    nc = tc.nc

    wpool = ctx.enter_context(tc.tile_pool(name="wpool", bufs=1))
    state_pool = ctx.enter_context(tc.tile_pool(name="state", bufs=1))

    # ---- weights resident in SBUF (leaf weights first: needed immediately)
    wleaf16_t = wpool.tile([128, 3, 3, 128], F16, name="wleaf16")
    nc.sync.dma_start(wleaf16_t[:], wleaf16_d[:].rearrange(
        "p (u kc m) -> p u kc m", u=3, kc=3))
    wleaf8_t = wpool.tile([128, 5, 2, 2, 128], F8, name="wleaf8")
    nc.sync.dma_start(wleaf8_t[:], wleaf8_d[:].rearrange(
        "p (u pr two m) -> p u pr two m", u=5, pr=2, two=2))
    wrec8_t = wpool.tile([128, 10, 3, 2, 128], F8, name="wrec8")
    wrec16_t = wpool.tile([128, 3, 5, 128], F16, name="wrec16")

    # ---- persistent state for levels 2..8: c16/h16 [128, 5, R/2] slots
    # (E0 E1 O0 O1 t2p), h8 [128, 3, 2, R/2] pairs ((E0,E1),(O0,O1),(t2p,0))
    ST = {}
    for lvl in range(2, 9):
        R = R_LVL[lvl]
        ST[lvl] = dict(
            c=state_pool.tile([128, 5, R // 2], F16, name=f"c_{lvl}"),
            h16=state_pool.tile([128, 5, R // 2], F16, name=f"h16_{lvl}"),
            h8=state_pool.tile([128, 3, 2, R // 2], F8, name=f"h8_{lvl}"),
            R=R)

    # leaf cells (per leaf block, 2 bufs) and L1 cells (1024 L1-cols each)
    LC = [dict(c=state_pool.tile([128, 5, LB // 2], F16, name=f"lc{i}"),
               h16=state_pool.tile([128, 5, LB // 2], F16, name=f"lh16{i}"),
               h8=state_pool.tile([128, 3, 2, LB // 2], F8, name=f"lh8{i}"),
               R=LB) for i in range(2)]
    C1 = [dict(c=state_pool.tile([128, 5, NF], F16, name=f"c1_{i}"),
               h16=state_pool.tile([128, 5, NF], F16, name=f"h16_1{i}"),
               h8=state_pool.tile([128, 3, 2, NF], F8, name=f"h8_1{i}"),
               R=2 * NF) for i in range(2)]

    # persistent x2 (fp16 leaf tail chunk with bias/zero rows)
    x2_t = [state_pool.tile([128, LB], F16, name=f"x2_{i}") for i in range(2)]

    def _pad_cell(cell):
        W = cell["R"] // 2
        nc.sync.dma_start(cell["h16"][44:64, 4, :], cons_d[0:20, :W])
        nc.sync.dma_start(cell["h16"][108:128, 4, :], cons_d[1:21, :W])
        nc.sync.dma_start(cell["c"][44:64, 4, :], cons_d[1:21, :W])
        nc.sync.dma_start(cell["c"][108:128, 4, :], cons_d[1:21, :W])
        nc.sync.dma_start(cell["h8"][44:64, 2, 0, :], cons8_d[0:20, :W])
        nc.sync.dma_start(cell["h8"][108:128, 2, 0, :], cons8_d[1:21, :W])
        # pair-2 slot 1: fully zero (matching weight slot is zero too)
        nc.sync.dma_start(cell["h8"][0:64, 2, 1, :], cons8_d[1:65, :W])
        nc.sync.dma_start(cell["h8"][64:128, 2, 1, :], cons8_d[1:65, :W])

    def _pad_x2():
        for i in range(2):
            nc.sync.dma_start(x2_t[i][44:128, :], cons_d[0:84, :LB])

    def _pad_dmas_early():
        for cell in LC + C1:
            _pad_cell(cell)

    def _pad_dmas_late():
        for lvl in range(2, 9):
            _pad_cell(ST[lvl])

    # ---- pools
    xpool = ctx.enter_context(tc.tile_pool(name="xpool", bufs=2))
    glpool = ctx.enter_context(tc.tile_pool(name="gl", bufs=2))
    gpool = ctx.enter_context(tc.tile_pool(name="g", bufs=2))
    pspool = ctx.enter_context(tc.tile_pool(name="ps", bufs=2, space="PSUM"))
    thpool = ctx.enter_context(tc.tile_pool(name="th", bufs=2))
    tpool = ctx.enter_context(tc.tile_pool(name="t", bufs=2))
    opool = ctx.enter_context(tc.tile_pool(name="o", bufs=1))

    # ================================================================ helpers
    def fp8_group(ps, units, rhs_pairs, PB):
        """Plain-fp8 DoubleRow GEMMs: 3 pair-instrs per unit."""
        for j, u in enumerate(units):
            for k in range(3):
                nc.tensor.matmul(ps[:, j * PB:(j + 1) * PB],
                                 wrec8_t[:, u, k, :, :], rhs_pairs[k],
                                 start=(k == 0), stop=(k == 2),
                                 perf_mode=DR)

    def fp16_group(ps, j0, units, wt, rhs_chunks, PB):
        nkc = len(rhs_chunks)
        for j, u in enumerate(units):
            for k in range(nkc):
                nc.tensor.matmul(ps[:, (j0 + j) * PB:(j0 + j + 1) * PB],
                                 wt[:, u, k, :], rhs_chunks[k],
                                 start=(k == 0), stop=(k == nkc - 1))

    def rec_block(lvl, q0, PB, prev, pq0, dst):
        """One recurrent block: cols q0:q0+PB at level lvl; children at
        E/O positions pq0:pq0+PB of `prev`; dst = cell dict or "root".

        Emits the A-part (GEMMs, gate ACTs, c-chain DVE) inline and returns
        the B-part (tanh-c ACTs + h stores) as a closure, so the caller can
        software-pipeline B behind the next block's A."""
        hw = PB // 2
        h8p = prev["h8"]
        rhs_pairs = [h8p[:, 0, :, pq0:pq0 + PB], h8p[:, 1, :, pq0:pq0 + PB],
                     h8p[:, 2, :, pq0:pq0 + PB]]
        h16p = prev["h16"]
        rhs16 = [h16p[:, 0, pq0:pq0 + PB], h16p[:, 1, pq0:pq0 + PB],
                 h16p[:, 4, pq0:pq0 + PB], h16p[:, 2, pq0:pq0 + PB],
                 h16p[:, 3, pq0:pq0 + PB]]

        G = gpool.tile([128, 13 * NF], F16, tag="G", name=f"G{lvl}")

        # G layout: [fL0 fL1 fR0 fR1 T9 i0 i1 o0 | o1 T8 | u0 u1 T12]
        # (sigma units contiguous at 0:10, tanh at 10:13)
        if PB > 256:
            ps1 = pspool.tile([128, 4 * NF], F32, tag="ps", name="ps1")
            fp8_group(ps1[:, :4 * PB], (0, 1, 2, 3), rhs_pairs, PB)
            ps2 = pspool.tile([128, 4 * NF], F32, tag="ps", name="ps2")
            fp8_group(ps2[:, :4 * PB], (4, 5, 6, 7), rhs_pairs, PB)
            nc.scalar.activation(G[:, 0:4 * PB], ps1[:, :4 * PB], SIG,
                                 scale=1.0 / SW)
            ps3 = pspool.tile([128, 4 * NF], F32, tag="ps", name="ps3")
            fp16_group(ps3, 0, (0, 1), wrec16_t, rhs16, PB)
            fp8_group(ps3[:, 2 * PB:4 * PB], (8, 9), rhs_pairs, PB)
            nc.scalar.activation(G[:, 4 * PB:8 * PB], ps2[:, :4 * PB], SIG,
                                 scale=1.0 / SW)
            ps4 = pspool.tile([128, 4 * NF], F32, tag="ps", name="ps4")
            fp16_group(ps4, 0, (2,), wrec16_t, rhs16, PB)
            # u01 tanh first: it gates the critical iu -> c -> tanh(c) chain
            nc.scalar.activation(G[:, 10 * PB:12 * PB], ps3[:, 0:2 * PB],
                                 TANH)
            nc.scalar.activation(G[:, 8 * PB:10 * PB], ps3[:, 2 * PB:4 * PB],
                                 SIG, scale=1.0 / SW)
            nc.scalar.activation(G[:, 12 * PB:13 * PB], ps4[:, 0:PB], TANH)
        else:
            # tail blocks: merged groups, fewer ACTs / PSUM round-trips
            n8 = 8 if PB == 256 else 10
            ps1 = pspool.tile([128, 4 * NF], F32, tag="ps", name="ps1")
            fp8_group(ps1[:, :n8 * PB], tuple(range(n8)), rhs_pairs, PB)
            ps2 = pspool.tile([128, 4 * NF], F32, tag="ps", name="ps2")
            fp16_group(ps2, 0, (0, 1, 2), wrec16_t, rhs16, PB)
            if n8 == 8:
                fp8_group(ps2[:, 3 * PB:5 * PB], (8, 9), rhs_pairs, PB)
            nc.scalar.activation(G[:, 0:n8 * PB], ps1[:, :n8 * PB], SIG,
                                 scale=1.0 / SW)
            nc.scalar.activation(G[:, 10 * PB:13 * PB], ps2[:, 0:3 * PB],
                                 TANH)
            if n8 == 8:
                nc.scalar.activation(G[:, 8 * PB:10 * PB],
                                     ps2[:, 3 * PB:5 * PB], SIG,
                                     scale=1.0 / SW)

        # ---- elementwise
        cp = prev["c"]
        c2p = cp[:, 4, pq0:pq0 + PB]           # [p, PB] (E@0:44, O@64:108)

        t1 = tpool.tile([128, 4, NF], F16, tag="t1", name="t1", bufs=1)
        t12a = tpool.tile([64, NF], F16, tag="t12a", name="t12a", bufs=1)
        t12b = tpool.tile([64, NF], F16, tag="t12b", name="t12b", bufs=1)
        fc = tpool.tile([128, 2, NF], F16, tag="fc", name="fc", bufs=1)
        fc2 = tpool.tile([64, NF], F16, tag="fc2", name="fc2", bufs=1)
        iu = tpool.tile([128, 2, NF], F16, tag="iu", name="iu", bufs=1)
        iu2 = tpool.tile([64, NF], F16, tag="iu2", name="iu2", bufs=1)

        g2 = G[:, 0:13 * PB].rearrange("p (u n) -> p u n", u=13)
        nc.vector.tensor_tensor(t1[:, :, :PB], g2[:, 0:4, :],
                                cp[:, 0:4, pq0:pq0 + PB], MUL)
        nc.vector.tensor_tensor(t12a[0:44, :PB], g2[0:44, GP_T9, :],
                                c2p[0:44], MUL)
        nc.vector.tensor_tensor(t12b[0:44, :PB], g2[64:108, GP_T9, :],
                                c2p[64:108], MUL)
        nc.vector.tensor_tensor(fc[:, :, :PB], t1[:, 0:2, :PB],
                                t1[:, 2:4, :PB], ADD)
        nc.vector.tensor_tensor(fc2[0:44, :PB], t12a[0:44, :PB],
                                t12b[0:44, :PB], ADD)
        nc.vector.tensor_tensor(iu[:, :, :PB], g2[:, GP_I:GP_I + 2, :],
                                g2[:, GP_U:GP_U + 2, :], MUL)
        nc.vector.tensor_tensor(iu2[0:44, :PB], g2[0:44, GP_T8, :],
                                g2[0:44, GP_T12, :], MUL)

        if dst == "root":
            ot = opool.tile([128, 6 * B_LOC], F16, name="ot")
            nc.sync.dma_start(ot[44:128, 2 * B_LOC:3 * B_LOC],
                              cons_d[0:84, B_LOC:2 * B_LOC])
            nc.sync.dma_start(ot[44:128, 5 * B_LOC:6 * B_LOC],
                              cons_d[0:84, B_LOC:2 * B_LOC])
            oc = ot[:, 0:2 * B_LOC].rearrange("p (c n) -> p c n", c=2)
            nc.vector.tensor_tensor(oc, iu[:, :, :PB], fc[:, :, :PB], ADD)
            nc.vector.tensor_tensor(ot[0:44, 2 * B_LOC:3 * B_LOC],
                                    iu2[0:44, :PB], fc2[0:44, :PB], ADD)
            tho = thpool.tile([128, 5, NF // 2], F16, tag="th", name="tho")
            nc.scalar.activation(tho[:, 0:2, :PB], oc, TANH)
            nc.scalar.activation(tho[64:108, 2, :PB],
                                 ot[0:44, 2 * B_LOC:3 * B_LOC], TANH)
            oh = ot[:, 3 * B_LOC:5 * B_LOC].rearrange("p (c n) -> p c n", c=2)
            nc.vector.tensor_tensor(oh, g2[:, GP_O:GP_O + 2, :],
                                    tho[:, 0:2, :PB], MUL)
            nc.vector.tensor_tensor(ot[0:44, 5 * B_LOC:6 * B_LOC],
                                    g2[64:108, GP_T8, :],
                                    tho[64:108, 2, :PB], MUL)
            nc.sync.dma_start(out_d[:, :], ot[:, :])
            return None

        qh = q0 // 2
        ct, h16t, h8t = dst["c"], dst["h16"], dst["h8"]
        # c store, E|O split (two ops: ISA allows at most 3 free dims)
        iu4 = iu[:, :, :PB].rearrange("p ch (m two b) -> p two ch m b",
                                      two=2, b=B_LOC)
        fc4 = fc[:, :, :PB].rearrange("p ch (m two b) -> p two ch m b",
                                      two=2, b=B_LOC)
        ce = ct[:, 0:2, qh:qh + hw].rearrange("p c (m b) -> p c m b", b=B_LOC)
        cod = ct[:, 2:4, qh:qh + hw].rearrange("p c (m b) -> p c m b", b=B_LOC)
        nc.vector.tensor_tensor(ce, iu4[:, 0], fc4[:, 0], ADD)
        nc.vector.tensor_tensor(cod, iu4[:, 1], fc4[:, 1], ADD)
        # c tail: interleaved at band [64:108], then E|O copies into the state
        tci = tpool.tile([128, NF], F16, tag="tci", name="tci")
        nc.vector.tensor_tensor(tci[64:108, :PB], iu2[0:44, :PB],
                                fc2[0:44, :PB], ADD)
        tcv = tci[64:108, :PB].rearrange("p (m two b) -> p m two b",
                                         two=2, b=B_LOC)
        c2o = ct[:, 4, qh:qh + hw].rearrange("p (m b) -> p m b", b=B_LOC)
        nc.vector.tensor_scalar_mul(c2o[0:44], tcv[:, :, 0, :], 1.0)
        nc.vector.tensor_scalar_mul(c2o[64:108], tcv[:, :, 1, :], 1.0)

        def finish():
            # tanh: main slots from the state, tail from the interleaved band
            th = thpool.tile([128, 5, NF // 2], F16, tag="th", name="th")
            nc.scalar.activation(th[:, 0:4, :hw], ct[:, 0:4, qh:qh + hw],
                                 TANH)
            th2i = tpool.tile([128, NF], F16, tag="th2i", name="th2i")
            nc.scalar.activation(th2i[64:108, :PB], tci[64:108, :PB], TANH)

            # h stores: h16 (GpSimd, off critical path) and h8 (DVE)
            o4 = g2[:, GP_O:GP_O + 2, :].rearrange(
                "p ch (m two b) -> p two ch m b", two=2, b=B_LOC)
            o2g = g2[64:108, GP_T8, :].rearrange("p (m two b) -> p m two b",
                                                 two=2, b=B_LOC)
            thr = th[:, :, :hw]
            the = thr[:, 0:2, :].rearrange("p ch (m b) -> p ch m b", b=B_LOC)
            tho_ = thr[:, 2:4, :].rearrange("p ch (m b) -> p ch m b", b=B_LOC)
            th2 = th2i[64:108, :PB].rearrange("p (m two b) -> p m two b",
                                              two=2, b=B_LOC)

            h16e = h16t[:, 0:2, qh:qh + hw].rearrange("p c (m b) -> p c m b",
                                                      b=B_LOC)
            h16o = h16t[:, 2:4, qh:qh + hw].rearrange("p c (m b) -> p c m b",
                                                      b=B_LOC)
            h16_2 = h16t[:, 4, qh:qh + hw].rearrange("p (m b) -> p m b",
                                                     b=B_LOC)
            nc.vector.tensor_tensor(h16e, o4[:, 0], the, MUL)
            nc.vector.tensor_tensor(h16o, o4[:, 1], tho_, MUL)
            nc.vector.tensor_tensor(h16_2[0:44], o2g[:, :, 0, :],
                                    th2[:, :, 0, :], MUL)
            nc.vector.tensor_tensor(h16_2[64:108], o2g[:, :, 1, :],
                                    th2[:, :, 1, :], MUL)

            h8e = h8t[:, 0, :, qh:qh + hw].rearrange("p c (m b) -> p c m b",
                                                     b=B_LOC)
            h8o = h8t[:, 1, :, qh:qh + hw].rearrange("p c (m b) -> p c m b",
                                                     b=B_LOC)
            nc.vector.tensor_tensor(h8e, o4[:, 0], the, MUL)
            nc.vector.tensor_tensor(h8o, o4[:, 1], tho_, MUL)
            h8_2 = h8t[:, 2, 0, qh:qh + hw].rearrange("p (m b) -> p m b",
                                                      b=B_LOC)
            nc.vector.tensor_tensor(h8_2[0:44], o2g[:, :, 0, :],
                                    th2[:, :, 0, :], MUL)
            nc.vector.tensor_tensor(h8_2[64:108], o2g[:, :, 1, :],
                                    th2[:, :, 1, :], MUL)

        return finish

    # ---------------------------------------------------------------- leaves
    def leaf_sub(x01, x2, x8, s, cell):
        """Leaf sub-chunk (512 cols): GEMM + gate ACTs + c-chain (A-part);
        returns the B-part closure.  Gl layout: [i0 i1 o0 o1 | T6 u0 u1 T7]"""
        n0 = s * NF
        hw = NF // 2
        qh = s * hw
        rhs16 = [x01[:, 0, n0:n0 + NF], x01[:, 1, n0:n0 + NF],
                 x2[:, n0:n0 + NF]]
        x8p = [x8[:, 0, :, n0:n0 + NF], x8[:, 1, :, n0:n0 + NF]]
        Gl = glpool.tile([128, 8 * NF], F16, tag="Gl", name="Gl")
        psA = pspool.tile([128, 4 * NF], F32, tag="ps", name="lpsA")
        for j, u in enumerate((0, 1, 2, 3)):
            for k in range(2):
                nc.tensor.matmul(psA[:, j * NF:(j + 1) * NF],
                                 wleaf8_t[:, u, k, :, :], x8p[k],
                                 start=(k == 0), stop=(k == 1), perf_mode=DR)
        psB = pspool.tile([128, 4 * NF], F32, tag="ps", name="lpsB")
        for k in range(2):
            nc.tensor.matmul(psB[:, 0:NF], wleaf8_t[:, 4, k, :, :], x8p[k],
                             start=(k == 0), stop=(k == 1), perf_mode=DR)
        fp16_group(psB, 1, (0, 1, 2), wleaf16_t, rhs16, NF)
        nc.scalar.activation(Gl[:, 0:4 * NF], psA[:, :], SIG, scale=1.0 / SW)
        nc.scalar.activation(Gl[:, 4 * NF:5 * NF], psB[:, 0:NF], SIG,
                             scale=1.0 / SW)
        nc.scalar.activation(Gl[:, 5 * NF:8 * NF], psB[:, NF:4 * NF], TANH)

        ct, h16t, h8t = cell["c"], cell["h16"], cell["h8"]
        g2 = Gl.rearrange("p (u n) -> p u n", u=8)
        i4 = g2[:, 0:2, :].rearrange("p ch (m two b) -> p two ch m b",
                                     two=2, b=B_LOC)
        u4 = g2[:, 5:7, :].rearrange("p ch (m two b) -> p two ch m b",
                                     two=2, b=B_LOC)

        ce = ct[:, 0:2, qh:qh + hw].rearrange("p c (m b) -> p c m b", b=B_LOC)
        co = ct[:, 2:4, qh:qh + hw].rearrange("p c (m b) -> p c m b", b=B_LOC)
        nc.vector.tensor_tensor(ce, i4[:, 0], u4[:, 0], MUL)
        nc.vector.tensor_tensor(co, i4[:, 1], u4[:, 1], MUL)
        # interleaved tail c at band [64:108], then E|O copies into the state
        tci = tpool.tile([128, NF], F16, tag="tci", name="ltci")
        nc.vector.tensor_tensor(tci[64:108, :NF], g2[0:44, 4, :],
                                g2[0:44, 7, :], MUL)
        tcv = tci[64:108, :NF].rearrange("p (m two b) -> p m two b",
                                         two=2, b=B_LOC)
        c2 = ct[:, 4, qh:qh + hw].rearrange("p (m b) -> p m b", b=B_LOC)
        nc.vector.tensor_scalar_mul(c2[0:44], tcv[:, :, 0, :], 1.0)
        nc.vector.tensor_scalar_mul(c2[64:108], tcv[:, :, 1, :], 1.0)

        def finish():
            o4 = g2[:, 2:4, :].rearrange("p ch (m two b) -> p two ch m b",
                                         two=2, b=B_LOC)
            o2g = g2[64:108, 4, :].rearrange("p (m two b) -> p m two b",
                                             two=2, b=B_LOC)
            th = thpool.tile([128, 5, NF // 2], F16, tag="th", name="lth")
            nc.scalar.activation(th[:, 0:4, :hw], ct[:, 0:4, qh:qh + hw],
                                 TANH)
            th2i = tpool.tile([128, NF], F16, tag="th2i", name="lth2i")
            nc.scalar.activation(th2i[64:108, :NF], tci[64:108, :NF], TANH)
            the = th[:, 0:2, :hw].rearrange("p c (m b) -> p c m b", b=B_LOC)
            tho_ = th[:, 2:4, :hw].rearrange("p c (m b) -> p c m b", b=B_LOC)
            th2 = th2i[64:108, :NF].rearrange("p (m two b) -> p m two b",
                                              two=2, b=B_LOC)

            h16e = h16t[:, 0:2, qh:qh + hw].rearrange("p c (m b) -> p c m b",
                                                      b=B_LOC)
            h16o = h16t[:, 2:4, qh:qh + hw].rearrange("p c (m b) -> p c m b",
                                                      b=B_LOC)
            h16_2 = h16t[:, 4, qh:qh + hw].rearrange("p (m b) -> p m b",
                                                     b=B_LOC)
            nc.vector.tensor_tensor(h16e, o4[:, 0], the, MUL)
            nc.vector.tensor_tensor(h16o, o4[:, 1], tho_, MUL)
            nc.vector.tensor_tensor(h16_2[0:44], o2g[:, :, 0, :],
                                    th2[:, :, 0, :], MUL)
            nc.vector.tensor_tensor(h16_2[64:108], o2g[:, :, 1, :],
                                    th2[:, :, 1, :], MUL)
            h8e = h8t[:, 0, :, qh:qh + hw].rearrange("p c (m b) -> p c m b",
                                                     b=B_LOC)
            h8o = h8t[:, 1, :, qh:qh + hw].rearrange("p c (m b) -> p c m b",
                                                     b=B_LOC)
            nc.vector.tensor_tensor(h8e, o4[:, 0], the, MUL)
            nc.vector.tensor_tensor(h8o, o4[:, 1], tho_, MUL)
            h8_2 = h8t[:, 2, 0, qh:qh + hw].rearrange("p (m b) -> p m b",
                                                      b=B_LOC)
            nc.vector.tensor_tensor(h8_2[0:44], o2g[:, :, 0, :],
                                    th2[:, :, 0, :], MUL)
            nc.vector.tensor_tensor(h8_2[64:108], o2g[:, :, 1, :],
                                    th2[:, :, 1, :], MUL)

        return finish

    # ================================================================ schedule
    # Software pipeline: emit A(block n) then B(block n-1); B(X) must precede
    # A(any consumer of X) -- guaranteed by the global block order below plus
    # explicit flushes at phase-B level boundaries.
    PEND = [None]

    def sched(b_fn):
        if PEND[0] is not None:
            PEND[0]()
        PEND[0] = b_fn

    def flush():
        if PEND[0] is not None:
            PEND[0]()
            PEND[0] = None

    n_blk = XCOLS // LB                       # 8 leaf blocks
    xtiles = {}

    def dma_x(j):
        c0 = j * LB
        x01 = xpool.tile([128, 2, LB], F16, tag="x01", name="x01")
        x8 = xpool.tile([128, 2, 2, LB], F8, tag="x8", name="x8")
        x2 = x2_t[j % 2]
        h = LB // 2
        for t in range(2):
            nc.sync.dma_start(x01[:, :, t * h:(t + 1) * h],
                              xt_d[:, :, c0 + t * h:c0 + (t + 1) * h])
            nc.sync.dma_start(
                x8[:, :, :, t * h:(t + 1) * h].rearrange(
                    "p pr two n -> p (pr two) n"),
                x8t_d[:, :, c0 + t * h:c0 + (t + 1) * h])
            nc.sync.dma_start(x2[0:44, t * h:(t + 1) * h],
                              x2t_d[:, c0 + t * h:c0 + (t + 1) * h])
        xtiles[j] = (x01, x2, x8)

    _pad_x2()
    dma_x(0)
    for blk in range(n_blk):
        x01, x2, x8 = xtiles.pop(blk)
        if blk + 1 < n_blk:
            dma_x(blk + 1)                    # prefetch next block's x
        if blk == 0:
            _pad_dmas_early()
            nc.sync.dma_start(wrec8_t[:], wrec8_d[:].rearrange(
                "p (u pr two m) -> p u pr two m", u=10, pr=3, two=2))
            nc.sync.dma_start(wrec16_t[:], wrec16_d[:].rearrange(
                "p (u kc m) -> p u kc m", u=3, kc=5))
        elif blk == 2:
            _pad_dmas_late()

        cell = LC[blk % 2]
        for s in range(2):
            sched(leaf_sub(x01, x2, x8, s, cell))
        if blk >= 1:
            j = blk - 1
            # L1 block j: output into C1 cell j//2 at col offset (j%2)*NF
            sched(rec_block(1, (j % 2) * NF, NF, LC[j % 2], 0,
                            C1[(j // 2) % 2]))
        if blk >= 3 and blk % 2 == 1:
            k = (blk - 3) // 2
            sched(rec_block(2, k * NF, NF, C1[k % 2], 0, ST[2]))
    # drain: l1(7), l2(3) -- l2(3) consumes l1(7) so flush in between
    sched(rec_block(1, NF, NF, LC[1], 0, C1[1]))
    flush()
    sched(rec_block(2, 3 * NF, NF, C1[1], 0, ST[2]))
    flush()

    # ---------------------------------------------------------------- phase B
    for lvl in range(3, 10):
        R = R_LVL[lvl]
        PB = min(NF, R)
        prev = ST[lvl - 1]
        for q0 in range(0, R, PB):
            b = rec_block(lvl, q0, PB, prev, q0,
                          ST[lvl] if lvl < 9 else "root")
            if b is not None:
                sched(b)
        flush()


# ---------------------------------------------------------------- runner
_CACHE = {}


def _get_program():
    if "nc" not in _CACHE:
        _CACHE["nc"] = build_program()
    return _CACHE["nc"]


def _host_inputs(inputs, Wfioux, b_fioux, Wiouh, Wfh):
    import ml_dtypes
    E4 = ml_dtypes.float8_e4m3fn
    wrec8, wrec16, wleaf8, wleaf16 = _pack_weights(
        np.asarray(Wfioux, np.float32), np.asarray(b_fioux, np.float32),
        np.asarray(Wiouh, np.float32), np.asarray(Wfh, np.float32))
    cons = np.zeros((84, 2 * LB), np.float16)
    cons[0, :] = 1.0
    cons8 = np.zeros((84, LB), np.float32)
    cons8[0, :] = 1.0
    cons8 = cons8.astype(E4)
    in_maps = []
    for core in range(N_CORES):
        x = np.asarray(inputs[core * B_LOC:(core + 1) * B_LOC, :N_LEAVES, :],
                       np.float32)
        xt_full = x.transpose(2, 1, 0).reshape(MEM, XCOLS)
        xt = np.ascontiguousarray(
            xt_full[0:256].reshape(2, 128, XCOLS).transpose(1, 0, 2)
        ).astype(np.float16)
        x2t = np.ascontiguousarray(xt_full[256:300]).astype(np.float16)
        x8t = np.zeros((128, 4, XCOLS), np.float32)
        x8t[:, 0, :] = xt_full[0:128]
        x8t[:, 1, :] = xt_full[128:256]
        x8t[0:44, 2, :] = xt_full[256:300]
        x8t[44, 2, :] = 1.0
        x8t = x8t.astype(E4)
        in_maps.append({"xt": xt, "x2t": x2t, "x8t": x8t, "wrec8": wrec8,
                        "wrec16": wrec16, "wleaf8": wleaf8,
                        "wleaf16": wleaf16, "cons": cons, "cons8": cons8})
    return in_maps


def kernel(inputs, Wfioux, b_fioux, Wiouh, Wfh, left_idx, right_idx, leaf_mask,
           _trace=False, _trace_dir=None):
    inputs = np.asarray(inputs, np.float32)
    assert _check_topology(left_idx, right_idx, leaf_mask), \
        "tree topology does not match the expected complete binary tree"

    in_maps = _host_inputs(inputs, Wfioux, b_fioux, Wiouh, Wfh)
    nc = _get_program()
    res = run_bass_kernel_spmd(nc, in_maps, list(range(N_CORES)),
                               trace=_trace, tmpdir=_trace_dir)

    root_c = np.zeros((B, MEM), np.float32)
    root_h = np.zeros((B, MEM), np.float32)
    for core in range(N_CORES):
        out = np.asarray(res.results[core]["out"], np.float32)  # [128, 96]
        sl = slice(core * B_LOC, (core + 1) * B_LOC)
        root_c[sl, 0:128] = out[:, 0:16].T
        root_c[sl, 128:256] = out[:, 16:32].T
        root_c[sl, 256:300] = out[0:44, 32:48].T
        root_h[sl, 0:128] = out[:, 48:64].T
        root_h[sl, 128:256] = out[:, 64:80].T
        root_h[sl, 256:300] = out[0:44, 80:96].T
    _CACHE["last_results"] = res
    return root_c, root_h
